# revision 3
# baseline (speedup 1.0000x reference)
"""DifferentialAttention Trainium2 kernel (8 NeuronCores, SPMD).

Sharding: 2 batches x 4 token-blocks = 8 cores. Each core computes
projections + layernorm for its 512 tokens, AllGathers K/V within its
4-core (same-batch) group, then computes attention + output projection
for its own query tokens. Host stitches per-core [D, 512] outputs.

Layout strategy: everything "transposed" (features on partitions, tokens
on free dim) so the whole chain q-proj -> scores -> AV -> out-proj needs
zero on-device transposes:
  - LN mean subtraction is folded into host-side column-centering of the
    (ternary-quantized) weight matrices; only the variance needs device
    work (a ones-matmul over the squared activations).
  - softmax runs without max-subtraction (scores are O(+-8); exp is safe
    in fp32) and the denominator comes for free as a 65th "ones" column
    appended to V.
  - lambda (a host-computable scalar from lambda_q/lambda_k) rides in as
    a [1,1] input tensor so the compiled program is input-independent.

Matmuls run as float32r (full-rate fp32 mode of the PE).
"""

import os
import sys
import types

for _p in ("/opt/trn_rl_repo",):
    if os.path.isdir(_p) and _p not in sys.path:
        sys.path.append(_p)

import numpy as np

import concourse.bass as bass
import concourse.tile as tile
from concourse import bacc, mybir
from concourse.bass_utils import run_bass_kernel_spmd


def _install_ntff_shim():
    """bass_utils imports antenv.axon_hooks when tracing under axon; the
    container antenv stub lacks it. Back it with the ctypes hook."""
    if "antenv.axon_hooks" in sys.modules:
        return
    try:
        from trn_agent_boot.trn_boot import _ntff_profile_via_ctypes

        hook = _ntff_profile_via_ctypes("/opt/axon/libaxon_pjrt.so")
    except Exception:
        hook = None
    mod = types.ModuleType("antenv.axon_hooks")
    mod.get_axon_ntff_profile_hook = lambda: hook
    sys.modules["antenv.axon_hooks"] = mod


_install_ntff_shim()

# ----- problem dims (hardcoded per spec) -----
B, T, D = 2, 2048, 1024
H, DH = 16, 64
CH = 2 * H * DH  # 2048
EPS = 1e-5
NCORES = 8
GS = 4  # cores per batch group
GROUPS = [[0, 1, 2, 3], [4, 5, 6, 7]]

F32 = mybir.dt.float32
F32R = mybir.dt.float32r

_PROG_CACHE: dict = {}


def _bcast_part(ap, n):
    """AP view replicating a 1-partition AP across n partitions (step 0)."""
    return bass.AP(tensor=ap.tensor, offset=ap.offset, ap=[[0, n]] + list(ap.ap)[1:])


def build_program(t_total=T, has_beta=False):
    """Build the per-core SPMD program. t_total is the per-batch sequence
    length (2048 for the real problem; smaller for simulation)."""
    TLOC = t_total // GS  # query tokens owned by this core
    KT = D // 128  # contraction tiles for projections
    CHT = CH // 128  # channel tiles of q/k
    NKC = t_total // 128  # key chunks
    TC = TLOC // 128  # token chunks (v projection)
    L = TLOC // 128  # 128-blocks per gather block
    HP = H // 2  # head pairs
    VW = H * 65  # augmented v width (64 + ones column per head)

    nc = bacc.Bacc("TRN2", target_bir_lowering=False, debug=False, num_devices=NCORES)

    xT = nc.dram_tensor("xT", [D, TLOC], F32R, kind="ExternalInput").ap()
    wq_t = nc.dram_tensor("wq_t", [D, CH], F32R, kind="ExternalInput").ap()
    wk_t = nc.dram_tensor("wk_t", [D, CH], F32R, kind="ExternalInput").ap()
    wv_t = nc.dram_tensor("wv_t", [D, D], F32R, kind="ExternalInput").ap()
    wo_t = nc.dram_tensor("wo_t", [D, D], F32R, kind="ExternalInput").ap()
    gs_q = nc.dram_tensor("gs_q", [128, CHT], F32, kind="ExternalInput").ap()
    gs_k = nc.dram_tensor("gs_k", [128, CHT], F32, kind="ExternalInput").ap()
    lam_in = nc.dram_tensor("lam", [1, 1], F32, kind="ExternalInput").ap()
    ones_mm_in = nc.dram_tensor("ones_mm", [128, 128], F32R, kind="ExternalInput").ap()
    ones_one_in = nc.dram_tensor("ones_one", [128, H], F32R, kind="ExternalInput").ap()
    if has_beta:
        bq_in = nc.dram_tensor("bq", [128, CHT], F32, kind="ExternalInput").ap()
        bk_in = nc.dram_tensor("bk", [128, CHT], F32, kind="ExternalInput").ap()
    yT = nc.dram_tensor("yT", [D, TLOC], F32, kind="ExternalOutput").ap()

    with tile.TileContext(nc) as tc:
        with (
            tc.tile_pool(name="const", bufs=1) as const,
            tc.tile_pool(name="dram", bufs=1, space="DRAM") as dram,
            tc.tile_pool(name="rdd_pool", bufs=4, space="DRAM") as rdd_pool,
        ):
            # constants + tiny inputs
            ones_ch = const.tile([128, 128], F32R)
            nc.sync.dma_start(ones_ch[:], ones_mm_in[:])
            ones_one = const.tile([128, H], F32R)
            nc.sync.dma_start(ones_one[:], ones_one_in[:])
            lam_sb = const.tile([1, 1], F32)
            nc.sync.dma_start(lam_sb[:], lam_in[:])
            gsq_sb = const.tile([128, CHT], F32)
            nc.sync.dma_start(gsq_sb[:], gs_q[:])
            gsk_sb = const.tile([128, CHT], F32)
            nc.sync.dma_start(gsk_sb[:], gs_k[:])
            if has_beta:
                bq_sb = const.tile([128, CHT], F32)
                nc.sync.dma_start(bq_sb[:], bq_in[:])
                bk_sb = const.tile([128, CHT], F32)
                nc.sync.dma_start(bk_sb[:], bk_in[:])

            # DRAM bounce/gather buffers
            kT_loc = dram.tile([CH, TLOC], F32R)
            kT_g = dram.tile([GS, CH, TLOC], F32R)
            vA_loc = dram.tile([TLOC, VW], F32R)
            vA_g = dram.tile([GS, TLOC, VW], F32R)

            qf_sb = None  # set in phase 1

            # ---------------- Phase 1: projections + LN + gathers -------
            with (
                tc.tile_pool(name="xp", bufs=1) as xp,
                tc.tile_pool(name="wstrip", bufs=KT) as wstrip,
                tc.tile_pool(name="prime_p", bufs=1) as prime_p,
                tc.tile_pool(name="sq_p", bufs=3) as sq_p,
                tc.tile_pool(name="stat_p", bufs=4) as stat_p,
                tc.tile_pool(name="ev_p", bufs=3) as ev_p,
                tc.tile_pool(name="qf_p", bufs=1) as qf_p,
                tc.tile_pool(name="vaug_p", bufs=2) as vaug_p,
                tc.tile_pool(name="pp", bufs=1, space="PSUM") as pp,
            ):
                xT_sb = xp.tile([128, KT, TLOC], F32R)
                nc.sync.dma_start(
                    xT_sb[:], xT.rearrange("(k p) t -> p k t", p=128)
                )

                def proj_ln(w_dram, gs_sb, beta_sb, out_cb, strip_tag):
                    # weight strips (streamed; pool cycles the slots)
                    strips = []
                    for j in range(KT):
                        ws = wstrip.tile(
                            [128, CH], F32R, tag="w", name=f"w_{strip_tag}{j}"
                        )
                        nc.sync.dma_start(
                            ws[:, 0:CH], w_dram[j * 128 : (j + 1) * 128, :]
                        )
                        strips.append(ws)
                    prime = prime_p.tile([128, CHT, TLOC], F32, tag="prime")
                    varps = pp.tile([128, TLOC], F32, tag="var", bufs=2)
                    for t in range(CHT):
                        ps = pp.tile([128, TLOC], F32, tag="proj", bufs=2)
                        for j in range(KT):
                            nc.tensor.matmul(
                                ps[:],
                                strips[j][:, t * 128 : (t + 1) * 128],
                                xT_sb[:, j, :],
                                start=(j == 0),
                                stop=(j == KT - 1),
                            )
                        nc.vector.tensor_copy(prime[:, t, :], ps[:])
                        sq = sq_p.tile([128, TLOC], F32R, tag="sq")
                        nc.vector.tensor_mul(sq[:], prime[:, t, :], prime[:, t, :])
                        nc.tensor.matmul(
                            varps[:],
                            ones_ch[:],
                            sq[:],
                            start=(t == 0),
                            stop=(t == CHT - 1),
                        )
                    veps = stat_p.tile([128, TLOC], F32, tag="veps")
                    nc.vector.tensor_scalar_add(veps[:], varps[:], EPS)
                    rec = stat_p.tile([128, TLOC], F32, tag="rec")
                    nc.vector.reciprocal(rec[:], veps[:])
                    rstd = stat_p.tile([128, TLOC], F32, tag="rstd")
                    nc.scalar.sqrt(rstd[:], rec[:])
                    for t in range(CHT):
                        out_ap = out_cb(t)
                        nc.vector.scalar_tensor_tensor(
                            out=out_ap,
                            in0=prime[:, t, :],
                            scalar=gs_sb[:, t : t + 1],
                            in1=rstd[:],
                            op0=mybir.AluOpType.mult,
                            op1=mybir.AluOpType.mult,
                        )
                        if beta_sb is not None:
                            nc.vector.tensor_scalar_add(
                                out_ap, out_ap, beta_sb[:, t : t + 1]
                            )

                # --- K (first: feeds the collective) ---
                def k_out(t):
                    kf = ev_p.tile([128, TLOC], F32R, tag="kf", name=f"kf{t}")
                    return kf[:]

                k_tiles = []

                def k_out_rec(t):
                    ap = k_out(t)
                    k_tiles.append((t, ap))
                    return ap

                proj_ln(wk_t, gsk_sb, bk_sb if has_beta else None, k_out_rec, "k")
                for t, ap in k_tiles:
                    nc.sync.dma_start(kT_loc[t * 128 : (t + 1) * 128, :], ap)
                nc.gpsimd.collective_compute(
                    "AllGather",
                    mybir.AluOpType.bypass,
                    replica_groups=GROUPS,
                    ins=[kT_loc[:]],
                    outs=[kT_g[:]],
                )

                # --- V projection (second collective) ---
                vstrips = []
                for j in range(KT):
                    vs = wstrip.tile([128, D], F32R, tag="w", name=f"w_v{j}")
                    nc.sync.dma_start(vs[:], wv_t[j * 128 : (j + 1) * 128, :])
                    vstrips.append(vs)
                for c in range(TC):
                    vps = pp.tile([128, D], F32, tag="vproj", bufs=2)
                    for j in range(KT):
                        for n in range(D // 512):
                            nc.tensor.matmul(
                                vps[:, n * 512 : (n + 1) * 512],
                                xT_sb[:, j, c * 128 : (c + 1) * 128],
                                vstrips[j][:, n * 512 : (n + 1) * 512],
                                start=(j == 0),
                                stop=(j == KT - 1),
                            )
                    vaug = vaug_p.tile([128, H, 65], F32R, tag="vaug")
                    nc.scalar.copy(
                        vaug[:, :, 0:64], vps[:].rearrange("p (h d) -> p h d", h=H)
                    )
                    nc.sync.dma_start(vaug[:, :, 64:65], ones_one[:])
                    nc.sync.dma_start(
                        vA_loc[c * 128 : (c + 1) * 128, :],
                        vaug.rearrange("p h d -> p (h d)"),
                    )
                nc.gpsimd.collective_compute(
                    "AllGather",
                    mybir.AluOpType.bypass,
                    replica_groups=GROUPS,
                    ins=[vA_loc[:]],
                    outs=[vA_g[:]],
                )

                # --- Q (stays resident in SBUF) ---
                qf_sb = qf_p.tile([128, CHT, TLOC], F32R)
                proj_ln(
                    wq_t,
                    gsq_sb,
                    bq_sb if has_beta else None,
                    lambda t: qf_sb[:, t, :],
                    "q",
                )

            # ---------------- Phase 2: attention ------------------------
            with (
                tc.tile_pool(name="kh_p", bufs=4) as kh_p,
                tc.tile_pool(name="vh_p", bufs=2) as vh_p,
                tc.tile_pool(name="pt_p", bufs=2) as pt_p,
                tc.tile_pool(name="o1_p", bufs=2) as o1_p,
                tc.tile_pool(name="rd_p", bufs=4) as rd_p,
                tc.tile_pool(name="rdb_p", bufs=4) as rdb_p,
                tc.tile_pool(name="attn_p", bufs=1) as attn_p,
                tc.tile_pool(name="scp", bufs=1, space="PSUM") as scp,
                tc.tile_pool(name="avp", bufs=4, space="PSUM") as avp,
            ):
                attn_sb = attn_p.tile([128, HP, TLOC], F32R)
                for hp in range(HP):
                    vh = vh_p.tile([128, NKC, 130], F32R, tag="vh")
                    for g in range(GS):
                        nc.sync.dma_start(
                            vh[:, g * L : (g + 1) * L, :],
                            vA_g[g, :, (2 * hp) * 65 : (2 * hp + 2) * 65].rearrange(
                                "(l p) c -> p l c", p=128
                            ),
                        )
                    o1 = o1_p.tile([128, TLOC], F32, tag="o1")
                    for b in range(2):
                        kh = kh_p.tile([128, t_total], F32R, tag="kh")
                        base = b * (CH // 2) + hp * 128
                        for g in range(GS):
                            nc.sync.dma_start(
                                kh[:, g * TLOC : (g + 1) * TLOC],
                                kT_g[g, base : base + 128, :],
                            )
                        avE = avp.tile([65, TLOC], F32, tag="av")
                        avO = avp.tile([65, TLOC], F32, tag="av")
                        qE = qf_sb[0:64, b * (CHT // 2) + hp, :]
                        qO = qf_sb[64:128, b * (CHT // 2) + hp, :]
                        for cg in range(NKC // 2):
                            sc = scp.tile([128, 2, 2, TLOC], F32, tag="sc")
                            for ci in range(2):
                                c = cg * 2 + ci
                                nc.tensor.matmul(
                                    sc[:, 0, ci, :],
                                    kh[0:64, c * 128 : (c + 1) * 128],
                                    qE,
                                    start=True,
                                    stop=True,
                                )
                                nc.tensor.matmul(
                                    sc[:, 1, ci, :],
                                    kh[64:128, c * 128 : (c + 1) * 128],
                                    qO,
                                    start=True,
                                    stop=True,
                                )
                            pt = pt_p.tile([128, 2, 2, TLOC], F32R, tag="pt")
                            nc.scalar.activation(
                                pt[:], sc[:], mybir.ActivationFunctionType.Exp
                            )
                            for ci in range(2):
                                c = cg * 2 + ci
                                nc.tensor.matmul(
                                    avE[:],
                                    vh[:, c, 0:65],
                                    pt[:, 0, ci, :],
                                    start=(c == 0),
                                    stop=(c == NKC - 1),
                                )
                                nc.tensor.matmul(
                                    avO[:],
                                    vh[:, c, 65:130],
                                    pt[:, 1, ci, :],
                                    start=(c == 0),
                                    stop=(c == NKC - 1),
                                )
                        for parity, av in ((0, avE), (1, avO)):
                            # denominator -> reciprocal (rebase to partition 0
                            # via a copy: DVE requires aligned SBUF operands)
                            rdc = rd_p.tile([1, TLOC], F32, tag="rdc")
                            nc.vector.tensor_copy(rdc[:], av[64:65, :])
                            rd = rd_p.tile([1, TLOC], F32, tag="rd")
                            nc.vector.reciprocal(rd[:], rdc[:])
                            if b == 1:
                                nc.vector.tensor_scalar_mul(
                                    rd[:], rd[:], lam_sb[0:1, 0:1]
                                )
                            rdd = rdd_pool.tile([1, TLOC], F32, tag="rdd")
                            nc.sync.dma_start(rdd[:], rd[:])
                            rows = slice(parity * 64, parity * 64 + 64)
                            rdb = rdb_p.tile([128, TLOC], F32, tag="rdb")
                            nc.sync.dma_start(rdb[rows, :], _bcast_part(rdd[:], 64))
                            if b == 0:
                                nc.vector.tensor_mul(
                                    o1[rows, :], av[0:64, :], rdb[rows, :]
                                )
                            else:
                                o2 = rdb_p.tile([128, TLOC], F32, tag="o2")
                                nc.vector.tensor_mul(
                                    o2[rows, :], av[0:64, :], rdb[rows, :]
                                )
                                nc.vector.tensor_sub(
                                    attn_sb[rows, hp, :], o1[rows, :], o2[rows, :]
                                )

            # ---------------- Phase 3: output projection ----------------
            with (
                tc.tile_pool(name="wo_p", bufs=KT) as wo_p,
                tc.tile_pool(name="ye_p", bufs=3) as ye_p,
                tc.tile_pool(name="yp", bufs=2, space="PSUM") as yp,
            ):
                wostrips = []
                for j in range(KT):
                    ws = wo_p.tile([128, D], F32R, tag="wo", name=f"w_o{j}")
                    nc.sync.dma_start(ws[:], wo_t[j * 128 : (j + 1) * 128, :])
                    wostrips.append(ws)
                for dt_ in range(D // 128):
                    yps = yp.tile([128, TLOC], F32, tag="y")
                    for j in range(KT):
                        nc.tensor.matmul(
                            yps[:],
                            wostrips[j][:, dt_ * 128 : (dt_ + 1) * 128],
                            attn_sb[:, j, :],
                            start=(j == 0),
                            stop=(j == KT - 1),
                        )
                    ye = ye_p.tile([128, TLOC], F32, tag="ye")
                    nc.scalar.copy(ye[:], yps[:])
                    nc.sync.dma_start(yT[dt_ * 128 : (dt_ + 1) * 128, :], ye[:])

    nc.compile()
    return nc


# ---------------- host-side preparation ----------------


def _quantize(W):
    W = np.asarray(W, dtype=np.float32)
    scale = np.clip(np.abs(W).mean(axis=1, keepdims=True), 1e-5, None)
    wq = np.clip(np.round(W / scale), -1.0, 1.0)
    return (wq * scale).astype(np.float32)


def prepare_inputs(
    x, Wq, Wk, Wv, Wo, lambda_q, lambda_k, qn_gamma, qn_beta, kn_gamma, kn_beta
):
    """Host prep: quantize + center weights, fold gamma/scale, per-core slices."""
    x = np.asarray(x, dtype=np.float32)
    t_total = x.shape[1]
    ch = 2 * H * DH
    cht = ch // 128
    tloc = t_total // GS

    Wq_e = _quantize(Wq)
    Wk_e = _quantize(Wk)
    Wv_e = _quantize(Wv)
    Wo_e = _quantize(Wo)
    # fold LN mean-subtraction into column-centered weights
    Wq_c = Wq_e - Wq_e.mean(axis=0, keepdims=True)
    Wk_c = Wk_e - Wk_e.mean(axis=0, keepdims=True)

    wq_t = np.ascontiguousarray(Wq_c.T)
    wk_t = np.ascontiguousarray(Wk_c.T)
    wv_t = np.ascontiguousarray(Wv_e.T)
    wo_t = np.ascontiguousarray(Wo_e.T)

    gq = np.asarray(qn_gamma, np.float32) * (DH**-0.5)
    gk = np.asarray(kn_gamma, np.float32)
    gs_q = np.ascontiguousarray(gq.reshape(cht, 128).T)
    gs_k = np.ascontiguousarray(gk.reshape(cht, 128).T)

    lam = np.clip(
        np.exp(np.asarray(lambda_q).mean() - np.asarray(lambda_k).mean()), 0.1, 2.0
    ).astype(np.float32)

    has_beta = bool(np.any(np.asarray(qn_beta)) or np.any(np.asarray(kn_beta)))
    common = {
        "wq_t": wq_t,
        "wk_t": wk_t,
        "wv_t": wv_t,
        "wo_t": wo_t,
        "gs_q": gs_q,
        "gs_k": gs_k,
        "lam": lam.reshape(1, 1),
        "ones_mm": np.full((128, 128), 1.0 / ch, np.float32),
        "ones_one": np.ones((128, H), np.float32),
    }
    if has_beta:
        bq = np.asarray(qn_beta, np.float32) * (DH**-0.5)
        bk = np.asarray(kn_beta, np.float32)
        common["bq"] = np.ascontiguousarray(bq.reshape(cht, 128).T)
        common["bk"] = np.ascontiguousarray(bk.reshape(cht, 128).T)

    in_maps = []
    for c in range(NCORES):
        b = c // GS
        ts = c % GS
        xt = np.ascontiguousarray(x[b, ts * tloc : (ts + 1) * tloc, :].T)
        in_maps.append({**common, "xT": xt})
    return in_maps, has_beta, t_total


def get_program(t_total=T, has_beta=False):
    key = (t_total, has_beta)
    if key not in _PROG_CACHE:
        _PROG_CACHE[key] = build_program(t_total, has_beta)
    return _PROG_CACHE[key]


def run(inputs, trace=False):
    """Run on hardware; returns (full_output, BassKernelResults)."""
    in_maps, has_beta, t_total = prepare_inputs(**inputs)
    nc = get_program(t_total, has_beta)
    res = run_bass_kernel_spmd(nc, in_maps, list(range(NCORES)), trace=trace)
    tloc = t_total // GS
    out = np.empty((B, t_total, D), dtype=np.float32)
    for c in range(NCORES):
        b = c // GS
        ts = c % GS
        out[b, ts * tloc : (ts + 1) * tloc, :] = res.results[c]["yT"].T
    return out, res


def kernel(**inputs) -> np.ndarray:
    out, _ = run(inputs, trace=False)
    return out


# revision 4
# speedup vs baseline: 1.5220x; 1.5220x over previous
"""DifferentialAttention Trainium2 kernel (8 NeuronCores, SPMD).

Sharding: 2 batches x 4 token-blocks = 8 cores. Each core computes
projections + layernorm for its 512 tokens, AllGathers K/V within its
4-core (same-batch) group, then computes attention + output projection
for its own query tokens. Host stitches per-core [D, 512] outputs.

Layout strategy: everything "transposed" (features on partitions, tokens
on free dim) so the whole chain q-proj -> scores -> AV -> out-proj needs
zero on-device transposes:
  - LN mean subtraction is folded into host-side column-centering of the
    (ternary-quantized) weight matrices; only the variance needs device
    work (a ones-matmul over the squared activations).
  - softmax runs without max-subtraction (scores are O(+-8); exp is safe)
    and the denominator comes for free as a 65th "ones" column appended
    to V.
  - lambda (a host-computable scalar from lambda_q/lambda_k) rides in as
    a [1,1] input tensor so the compiled program is input-independent.

Collectives are ordered V, K-branch0, K-branch1 and the attention loop
runs branch-outer so branch-0 attention overlaps the branch-1 gather.

MM_DT picks the matmul operand dtype: float16 (default; full-rate PE,
values here are well within fp16 range), bfloat16, or float32r.
"""

import os
import sys
import types

for _p in ("/opt/trn_rl_repo",):
    if os.path.isdir(_p) and _p not in sys.path:
        sys.path.append(_p)

import numpy as np

import concourse.bass as bass
import concourse.tile as tile
from concourse import bacc, mybir
from concourse.bass_utils import run_bass_kernel_spmd


def _install_ntff_shim():
    """bass_utils imports antenv.axon_hooks when tracing under axon; the
    container antenv stub lacks it. Back it with the ctypes hook."""
    if "antenv.axon_hooks" in sys.modules:
        return
    try:
        from trn_agent_boot.trn_boot import _ntff_profile_via_ctypes

        hook = _ntff_profile_via_ctypes("/opt/axon/libaxon_pjrt.so")
    except Exception:
        hook = None
    mod = types.ModuleType("antenv.axon_hooks")
    mod.get_axon_ntff_profile_hook = lambda: hook
    sys.modules["antenv.axon_hooks"] = mod


_install_ntff_shim()

# ----- problem dims (hardcoded per spec) -----
B, T, D = 2, 2048, 1024
H, DH = 16, 64
CH = 2 * H * DH  # 2048
EPS = 1e-5
NCORES = 8
GS = 4  # cores per batch group
GROUPS = [[0, 1, 2, 3], [4, 5, 6, 7]]

F32 = mybir.dt.float32
MM_DT = "f16"  # "f16" | "bf16" | "f32r"
_DT_MAP = {
    "f16": mybir.dt.float16,
    "bf16": mybir.dt.bfloat16,
    "f32r": mybir.dt.float32r,
}

_PROG_CACHE: dict = {}


def _bcast_part(ap, n):
    """AP view replicating a 1-partition AP across n partitions (step 0)."""
    return bass.AP(tensor=ap.tensor, offset=ap.offset, ap=[[0, n]] + list(ap.ap)[1:])


def build_program(t_total=T, has_beta=False, mm_dt=MM_DT):
    """Build the per-core SPMD program. t_total is the per-batch sequence
    length (2048 for the real problem; smaller for simulation)."""
    TLOC = t_total // GS  # query tokens owned by this core
    KT = D // 128  # contraction tiles for projections
    CHT = CH // 128  # channel tiles of q/k
    NKC = t_total // 128  # key chunks
    TC = TLOC // 128  # token chunks (v projection)
    L = TLOC // 128  # 128-blocks per gather block
    HP = H // 2  # head pairs
    VW = H * 65  # augmented v width (64 + ones column per head)
    DT = _DT_MAP[mm_dt]

    nc = bacc.Bacc("TRN2", target_bir_lowering=False, debug=False, num_devices=NCORES)

    xT = nc.dram_tensor("xT", [D, TLOC], DT, kind="ExternalInput").ap()
    wq_t = nc.dram_tensor("wq_t", [D, CH], DT, kind="ExternalInput").ap()
    wk_t = nc.dram_tensor("wk_t", [D, CH], DT, kind="ExternalInput").ap()
    wv_t = nc.dram_tensor("wv_t", [D, D], DT, kind="ExternalInput").ap()
    wo_t = nc.dram_tensor("wo_t", [D, D], DT, kind="ExternalInput").ap()
    gs_q = nc.dram_tensor("gs_q", [128, CHT], F32, kind="ExternalInput").ap()
    gs_k = nc.dram_tensor("gs_k", [128, CHT], F32, kind="ExternalInput").ap()
    lam_in = nc.dram_tensor("lam", [1, 1], F32, kind="ExternalInput").ap()
    ones_mm_in = nc.dram_tensor("ones_mm", [128, 128], DT, kind="ExternalInput").ap()
    ones_one_in = nc.dram_tensor("ones_one", [128, H], DT, kind="ExternalInput").ap()
    if has_beta:
        bq_in = nc.dram_tensor("bq", [128, CHT], F32, kind="ExternalInput").ap()
        bk_in = nc.dram_tensor("bk", [128, CHT], F32, kind="ExternalInput").ap()
    yT = nc.dram_tensor("yT", [D, TLOC], F32, kind="ExternalOutput").ap()

    with tile.TileContext(nc) as tc:
        with (
            tc.tile_pool(name="const", bufs=1) as const,
            tc.tile_pool(name="dram", bufs=1, space="DRAM") as dram,
            tc.tile_pool(name="rdd_pool", bufs=4, space="DRAM") as rdd_pool,
        ):
            # constants + tiny inputs
            ones_ch = const.tile([128, 128], DT)
            nc.sync.dma_start(ones_ch[:], ones_mm_in[:])
            ones_one = const.tile([128, H], DT)
            nc.sync.dma_start(ones_one[:], ones_one_in[:])
            lam_sb = const.tile([1, 1], F32)
            nc.sync.dma_start(lam_sb[:], lam_in[:])
            gsq_sb = const.tile([128, CHT], F32)
            nc.sync.dma_start(gsq_sb[:], gs_q[:])
            gsk_sb = const.tile([128, CHT], F32)
            nc.sync.dma_start(gsk_sb[:], gs_k[:])
            bq_sb = bk_sb = None
            if has_beta:
                bq_sb = const.tile([128, CHT], F32)
                nc.sync.dma_start(bq_sb[:], bq_in[:])
                bk_sb = const.tile([128, CHT], F32)
                nc.sync.dma_start(bk_sb[:], bk_in[:])

            # DRAM bounce/gather buffers
            kT_loc = dram.tile([CH, TLOC], DT)
            kT_g0 = dram.tile([GS, CH // 2, TLOC], DT)
            kT_g1 = dram.tile([GS, CH // 2, TLOC], DT)
            vA_loc = dram.tile([TLOC, VW], DT)
            vA_g = dram.tile([GS, TLOC, VW], DT)

            # ---------------- Phase 1: projections + LN + gathers -------
            with (
                tc.tile_pool(name="xp", bufs=1) as xp,
                tc.tile_pool(name="wv_p", bufs=KT) as wv_p,
                tc.tile_pool(name="wk_p", bufs=KT) as wk_p,
                tc.tile_pool(name="wq_p", bufs=KT) as wq_p,
                tc.tile_pool(name="prime_p", bufs=1) as prime_p,
                tc.tile_pool(name="sq_p", bufs=3) as sq_p,
                tc.tile_pool(name="stat_p", bufs=4) as stat_p,
                tc.tile_pool(name="ev_p", bufs=3) as ev_p,
                tc.tile_pool(name="qf_p", bufs=1) as qf_p,
                tc.tile_pool(name="vaug_p", bufs=2) as vaug_p,
                tc.tile_pool(name="pp", bufs=1, space="PSUM") as pp,
            ):
                xT_sb = xp.tile([128, KT, TLOC], DT)
                nc.sync.dma_start(xT_sb[:], xT.rearrange("(k p) t -> p k t", p=128))

                # --- V projection first (its gather unblocks attention) ---
                vstrips = []
                for j in range(KT):
                    vs = wv_p.tile([128, D], DT, tag="wv", name=f"w_v{j}")
                    nc.sync.dma_start(vs[:], wv_t[j * 128 : (j + 1) * 128, :])
                    vstrips.append(vs)
                for c in range(TC):
                    vps = pp.tile([128, D], F32, tag="vproj", bufs=2)
                    for j in range(KT):
                        for n in range(D // 512):
                            nc.tensor.matmul(
                                vps[:, n * 512 : (n + 1) * 512],
                                xT_sb[:, j, c * 128 : (c + 1) * 128],
                                vstrips[j][:, n * 512 : (n + 1) * 512],
                                start=(j == 0),
                                stop=(j == KT - 1),
                            )
                    vaug = vaug_p.tile([128, H, 65], DT, tag="vaug")
                    nc.scalar.copy(
                        vaug[:, :, 0:64], vps[:].rearrange("p (h d) -> p h d", h=H)
                    )
                    nc.sync.dma_start(vaug[:, :, 64:65], ones_one[:])
                    nc.sync.dma_start(
                        vA_loc[c * 128 : (c + 1) * 128, :],
                        vaug.rearrange("p h d -> p (h d)"),
                    )
                nc.gpsimd.collective_compute(
                    "AllGather",
                    mybir.AluOpType.bypass,
                    replica_groups=GROUPS,
                    ins=[vA_loc[:]],
                    outs=[vA_g[:]],
                )

                def proj_ln(w_pool, w_dram, gs_sb, beta_sb, out_cb, strip_tag, wtag):
                    strips = []
                    for j in range(KT):
                        ws = w_pool.tile(
                            [128, CH], DT, tag=wtag, name=f"w_{strip_tag}{j}"
                        )
                        nc.sync.dma_start(
                            ws[:, 0:CH], w_dram[j * 128 : (j + 1) * 128, :]
                        )
                        strips.append(ws)
                    prime = prime_p.tile([128, CHT, TLOC], F32, tag="prime")
                    varps = pp.tile([128, TLOC], F32, tag="var", bufs=2)
                    for t in range(CHT):
                        ps = pp.tile([128, TLOC], F32, tag="proj", bufs=2)
                        for j in range(KT):
                            nc.tensor.matmul(
                                ps[:],
                                strips[j][:, t * 128 : (t + 1) * 128],
                                xT_sb[:, j, :],
                                start=(j == 0),
                                stop=(j == KT - 1),
                            )
                        nc.vector.tensor_copy(prime[:, t, :], ps[:])
                        sq = sq_p.tile([128, TLOC], DT, tag="sq")
                        nc.scalar.square(sq[:], prime[:, t, :])
                        nc.tensor.matmul(
                            varps[:],
                            ones_ch[:],
                            sq[:],
                            start=(t == 0),
                            stop=(t == CHT - 1),
                        )
                    veps = stat_p.tile([128, TLOC], F32, tag="veps")
                    nc.vector.tensor_scalar_add(veps[:], varps[:], EPS)
                    rec = stat_p.tile([128, TLOC], F32, tag="rec")
                    nc.vector.reciprocal(rec[:], veps[:])
                    rstd = stat_p.tile([128, TLOC], F32, tag="rstd")
                    nc.scalar.sqrt(rstd[:], rec[:])
                    for t in range(CHT):
                        out_ap = out_cb(t)
                        nc.vector.scalar_tensor_tensor(
                            out=out_ap,
                            in0=prime[:, t, :],
                            scalar=gs_sb[:, t : t + 1],
                            in1=rstd[:],
                            op0=mybir.AluOpType.mult,
                            op1=mybir.AluOpType.mult,
                        )
                        if beta_sb is not None:
                            nc.vector.tensor_scalar_add(
                                out_ap, out_ap, beta_sb[:, t : t + 1]
                            )

                # --- K projection + per-branch gathers ---
                k_tiles = {}

                def k_out(t):
                    kf = ev_p.tile([128, TLOC], DT, tag="kf", name=f"kf{t}")
                    k_tiles[t] = kf
                    return kf[:]

                proj_ln(wk_p, wk_t, gsk_sb, bk_sb, k_out, "k", "wk")
                for t, kf in k_tiles.items():
                    nc.sync.dma_start(kT_loc[t * 128 : (t + 1) * 128, :], kf[:])
                nc.gpsimd.collective_compute(
                    "AllGather",
                    mybir.AluOpType.bypass,
                    replica_groups=GROUPS,
                    ins=[kT_loc[0 : CH // 2, :]],
                    outs=[kT_g0[:]],
                )
                nc.gpsimd.collective_compute(
                    "AllGather",
                    mybir.AluOpType.bypass,
                    replica_groups=GROUPS,
                    ins=[kT_loc[CH // 2 : CH, :]],
                    outs=[kT_g1[:]],
                )

                # --- Q (stays resident in SBUF) ---
                qf_sb = qf_p.tile([128, CHT, TLOC], DT)
                proj_ln(
                    wq_p, wq_t, gsq_sb, bq_sb, lambda t: qf_sb[:, t, :], "q", "wq"
                )

            # ---------------- Phase 2: attention ------------------------
            with (
                tc.tile_pool(name="kh_p", bufs=4) as kh_p,
                tc.tile_pool(name="vh_p", bufs=3) as vh_p,
                tc.tile_pool(name="pt_p", bufs=4) as pt_p,
                tc.tile_pool(name="o1_p", bufs=HP) as o1_p,
                tc.tile_pool(name="rd_p", bufs=4) as rd_p,
                tc.tile_pool(name="rdb_p", bufs=4) as rdb_p,
                tc.tile_pool(name="attn_p", bufs=1) as attn_p,
                tc.tile_pool(name="scp", bufs=3, space="PSUM") as scp,
                tc.tile_pool(name="avp", bufs=2, space="PSUM") as avp,
            ):
                attn_sb = attn_p.tile([128, HP, TLOC], DT)
                o1_tiles = {}
                for b in range(2):
                    kT_gb = kT_g0 if b == 0 else kT_g1
                    for hp in range(HP):
                        vh = vh_p.tile([128, NKC, 130], DT, tag="vh")
                        for g in range(GS):
                            nc.sync.dma_start(
                                vh[:, g * L : (g + 1) * L, :],
                                vA_g[
                                    g, :, (2 * hp) * 65 : (2 * hp + 2) * 65
                                ].rearrange("(l p) c -> p l c", p=128),
                            )
                        kh = kh_p.tile([128, t_total], DT, tag="kh")
                        for g in range(GS):
                            nc.sync.dma_start(
                                kh[:, g * TLOC : (g + 1) * TLOC],
                                kT_gb[g, hp * 128 : (hp + 1) * 128, :],
                            )
                        if b == 0:
                            o1 = o1_p.tile([128, TLOC], F32, tag="o1", name=f"o1_{hp}")
                            o1_tiles[hp] = o1
                        else:
                            o1 = o1_tiles[hp]
                        avE = avp.tile([65, TLOC], F32, tag="av")
                        avO = avp.tile([65, TLOC], F32, tag="av")
                        qE = qf_sb[0:64, b * (CHT // 2) + hp, :]
                        qO = qf_sb[64:128, b * (CHT // 2) + hp, :]
                        for c in range(NKC):
                            sc = scp.tile([128, 2, TLOC], F32, tag="sc")
                            nc.tensor.matmul(
                                sc[:, 0, :],
                                kh[0:64, c * 128 : (c + 1) * 128],
                                qE,
                                start=True,
                                stop=True,
                            )
                            nc.tensor.matmul(
                                sc[:, 1, :],
                                kh[64:128, c * 128 : (c + 1) * 128],
                                qO,
                                start=True,
                                stop=True,
                            )
                            pt = pt_p.tile([128, 2, TLOC], DT, tag="pt")
                            nc.scalar.activation(
                                pt[:], sc[:], mybir.ActivationFunctionType.Exp
                            )
                            nc.tensor.matmul(
                                avE[:],
                                vh[:, c, 0:65],
                                pt[:, 0, :],
                                start=(c == 0),
                                stop=(c == NKC - 1),
                            )
                            nc.tensor.matmul(
                                avO[:],
                                vh[:, c, 65:130],
                                pt[:, 1, :],
                                start=(c == 0),
                                stop=(c == NKC - 1),
                            )
                        for parity, av in ((0, avE), (1, avO)):
                            # denominator -> reciprocal (rebased to partition 0)
                            rdc = rd_p.tile([1, TLOC], F32, tag="rdc")
                            nc.vector.tensor_copy(rdc[:], av[64:65, :])
                            rd = rd_p.tile([1, TLOC], F32, tag="rd")
                            nc.vector.reciprocal(rd[:], rdc[:])
                            if b == 1:
                                nc.vector.tensor_scalar_mul(
                                    rd[:], rd[:], lam_sb[0:1, 0:1]
                                )
                            rdd = rdd_pool.tile([1, TLOC], F32, tag="rdd")
                            nc.sync.dma_start(rdd[:], rd[:])
                            rows = slice(parity * 64, parity * 64 + 64)
                            rdb = rdb_p.tile([128, TLOC], F32, tag="rdb")
                            nc.sync.dma_start(rdb[rows, :], _bcast_part(rdd[:], 64))
                            if b == 0:
                                nc.vector.tensor_mul(
                                    o1[rows, :], av[0:64, :], rdb[rows, :]
                                )
                            else:
                                o2 = rdb_p.tile([128, TLOC], F32, tag="o2")
                                nc.vector.tensor_mul(
                                    o2[rows, :], av[0:64, :], rdb[rows, :]
                                )
                                nc.vector.tensor_sub(
                                    attn_sb[rows, hp, :], o1[rows, :], o2[rows, :]
                                )

            # ---------------- Phase 3: output projection ----------------
            with (
                tc.tile_pool(name="wo_p", bufs=KT) as wo_p,
                tc.tile_pool(name="ye_p", bufs=3) as ye_p,
                tc.tile_pool(name="yp", bufs=2, space="PSUM") as yp,
            ):
                wostrips = []
                for j in range(KT):
                    ws = wo_p.tile([128, D], DT, tag="wo", name=f"w_o{j}")
                    nc.sync.dma_start(ws[:], wo_t[j * 128 : (j + 1) * 128, :])
                    wostrips.append(ws)
                for dt_ in range(D // 128):
                    yps = yp.tile([128, TLOC], F32, tag="y")
                    for j in range(KT):
                        nc.tensor.matmul(
                            yps[:],
                            wostrips[j][:, dt_ * 128 : (dt_ + 1) * 128],
                            attn_sb[:, j, :],
                            start=(j == 0),
                            stop=(j == KT - 1),
                        )
                    ye = ye_p.tile([128, TLOC], F32, tag="ye")
                    nc.scalar.copy(ye[:], yps[:])
                    nc.sync.dma_start(yT[dt_ * 128 : (dt_ + 1) * 128, :], ye[:])

    nc.compile()
    return nc


# ---------------- host-side preparation ----------------


def _quantize(W):
    W = np.asarray(W, dtype=np.float32)
    scale = np.clip(np.abs(W).mean(axis=1, keepdims=True), 1e-5, None)
    wq = np.clip(np.round(W / scale), -1.0, 1.0)
    return (wq * scale).astype(np.float32)


def prepare_inputs(
    x, Wq, Wk, Wv, Wo, lambda_q, lambda_k, qn_gamma, qn_beta, kn_gamma, kn_beta,
    mm_dt=MM_DT,
):
    """Host prep: quantize + center weights, fold gamma/scale, per-core slices."""
    np_dt = mybir.dt.np(_DT_MAP[mm_dt])
    x = np.asarray(x, dtype=np.float32)
    t_total = x.shape[1]
    ch = 2 * H * DH
    cht = ch // 128
    tloc = t_total // GS

    Wq_e = _quantize(Wq)
    Wk_e = _quantize(Wk)
    Wv_e = _quantize(Wv)
    Wo_e = _quantize(Wo)
    # fold LN mean-subtraction into column-centered weights
    Wq_c = Wq_e - Wq_e.mean(axis=0, keepdims=True)
    Wk_c = Wk_e - Wk_e.mean(axis=0, keepdims=True)

    wq_t = np.ascontiguousarray(Wq_c.T).astype(np_dt)
    wk_t = np.ascontiguousarray(Wk_c.T).astype(np_dt)
    wv_t = np.ascontiguousarray(Wv_e.T).astype(np_dt)
    wo_t = np.ascontiguousarray(Wo_e.T).astype(np_dt)

    gq = np.asarray(qn_gamma, np.float32) * (DH**-0.5)
    gk = np.asarray(kn_gamma, np.float32)
    gs_q = np.ascontiguousarray(gq.reshape(cht, 128).T)
    gs_k = np.ascontiguousarray(gk.reshape(cht, 128).T)

    lam = np.clip(
        np.exp(np.asarray(lambda_q).mean() - np.asarray(lambda_k).mean()), 0.1, 2.0
    ).astype(np.float32)

    has_beta = bool(np.any(np.asarray(qn_beta)) or np.any(np.asarray(kn_beta)))
    common = {
        "wq_t": wq_t,
        "wk_t": wk_t,
        "wv_t": wv_t,
        "wo_t": wo_t,
        "gs_q": gs_q,
        "gs_k": gs_k,
        "lam": lam.reshape(1, 1),
        "ones_mm": np.full((128, 128), 1.0 / ch, np_dt),
        "ones_one": np.ones((128, H), np_dt),
    }
    if has_beta:
        bq = np.asarray(qn_beta, np.float32) * (DH**-0.5)
        bk = np.asarray(kn_beta, np.float32)
        common["bq"] = np.ascontiguousarray(bq.reshape(cht, 128).T)
        common["bk"] = np.ascontiguousarray(bk.reshape(cht, 128).T)

    in_maps = []
    for c in range(NCORES):
        b = c // GS
        ts = c % GS
        xt = np.ascontiguousarray(x[b, ts * tloc : (ts + 1) * tloc, :].T).astype(np_dt)
        in_maps.append({**common, "xT": xt})
    return in_maps, has_beta, t_total


def get_program(t_total=T, has_beta=False, mm_dt=MM_DT):
    key = (t_total, has_beta, mm_dt)
    if key not in _PROG_CACHE:
        _PROG_CACHE[key] = build_program(t_total, has_beta, mm_dt)
    return _PROG_CACHE[key]


def run(inputs, trace=False, mm_dt=MM_DT):
    """Run on hardware; returns (full_output, BassKernelResults)."""
    in_maps, has_beta, t_total = prepare_inputs(**inputs, mm_dt=mm_dt)
    nc = get_program(t_total, has_beta, mm_dt)
    res = run_bass_kernel_spmd(nc, in_maps, list(range(NCORES)), trace=trace)
    tloc = t_total // GS
    out = np.empty((B, t_total, D), dtype=np.float32)
    for c in range(NCORES):
        b = c // GS
        ts = c % GS
        out[b, ts * tloc : (ts + 1) * tloc, :] = res.results[c]["yT"].T
    return out, res


def kernel(**inputs) -> np.ndarray:
    out, _ = run(inputs, trace=False)
    return out


# revision 7
# speedup vs baseline: 1.5850x; 1.0414x over previous
"""DifferentialAttention Trainium2 kernel (8 NeuronCores, SPMD).

Sharding: 2 batches x 4 token-blocks = 8 cores. Each core computes
projections + layernorm for its 512 tokens, AllGathers K/V within its
4-core (same-batch) group, then computes attention + output projection
for its own query tokens. Host stitches per-core [D, 512] outputs.

Layout strategy: everything "transposed" (features on partitions, tokens
on free dim) so the whole chain q-proj -> scores -> AV -> out-proj needs
zero on-device transposes:
  - LN mean subtraction is folded into host-side column-centering of the
    (ternary-quantized) weight matrices, and gamma is folded into the
    weight rows; only the variance needs device work (a matmul of the
    squared activations against a per-channel 1/(CH*gamma^2) stationary).
  - K's 1/std never touches K on device: it rides as the per-partition
    `scale` operand of the softmax exp (scoresT has keys on partitions),
    so raw centered K can be gathered as soon as its projection finishes.
  - softmax runs without max-subtraction (scores are O(+-8); exp is safe)
    and the denominator comes for free as a 65th "ones" column appended
    to V.
  - lambda (a host-computable scalar) rides in as a [1,1] input tensor so
    the compiled program is input-independent.

Attention is branch-outer, and each (branch, head-pair) runs all 16
score-matmuls + exps first, then all 32 AV matmuls — the PE's in-order
queue never waits on the ScalarE exp stream.

MM_DT picks the matmul operand dtype: float16 (default), bfloat16, or
float32r.
"""

import os
import sys
import types

for _p in ("/opt/trn_rl_repo",):
    if os.path.isdir(_p) and _p not in sys.path:
        sys.path.append(_p)

import numpy as np

import concourse.bass as bass
import concourse.tile as tile
from concourse import bacc, mybir
from concourse.bass_utils import run_bass_kernel_spmd


def _install_ntff_shim():
    """bass_utils imports antenv.axon_hooks when tracing under axon; the
    container antenv stub lacks it. Back it with the ctypes hook."""
    if "antenv.axon_hooks" in sys.modules:
        return
    try:
        from trn_agent_boot.trn_boot import _ntff_profile_via_ctypes

        hook = _ntff_profile_via_ctypes("/opt/axon/libaxon_pjrt.so")
    except Exception:
        hook = None
    mod = types.ModuleType("antenv.axon_hooks")
    mod.get_axon_ntff_profile_hook = lambda: hook
    sys.modules["antenv.axon_hooks"] = mod


_install_ntff_shim()

# ----- problem dims (hardcoded per spec) -----
B, T, D = 2, 2048, 1024
H, DH = 16, 64
CH = 2 * H * DH  # 2048
EPS = 1e-5
NCORES = 8
GS = 4  # cores per batch group
GROUPS = [[0, 1, 2, 3], [4, 5, 6, 7]]

F32 = mybir.dt.float32
MM_DT = "f16"  # "f16" | "bf16" | "f32r"
_DT_MAP = {
    "f16": mybir.dt.float16,
    "bf16": mybir.dt.bfloat16,
    "f32r": mybir.dt.float32r,
}

_PROG_CACHE: dict = {}


def _bcast_part(ap, n):
    """AP view replicating a 1-partition AP across n partitions (step 0)."""
    return bass.AP(tensor=ap.tensor, offset=ap.offset, ap=[[0, n]] + list(ap.ap)[1:])


def build_program(t_total=T, has_beta=False, mm_dt=MM_DT):
    """Build the per-core SPMD program. t_total is the per-batch sequence
    length (2048 for the real problem; smaller for simulation)."""
    TLOC = t_total // GS  # query tokens owned by this core
    KT = D // 128  # contraction tiles for projections
    CHT = CH // 128  # channel tiles of q/k
    NKC = t_total // 128  # key chunks
    TC = TLOC // 128  # token chunks (v projection)
    L = TLOC // 128  # 128-blocks per gather block
    HP = H // 2  # head pairs
    VW = H * 65  # augmented v width (64 + ones column per head)
    DT = _DT_MAP[mm_dt]
    fast_k = not has_beta  # beta on K forces the on-device LN-apply path

    nc = bacc.Bacc("TRN2", target_bir_lowering=False, debug=False, num_devices=NCORES)

    xT = nc.dram_tensor("xT", [D, TLOC], DT, kind="ExternalInput").ap()
    wq_t = nc.dram_tensor("wq_t", [D, CH], DT, kind="ExternalInput").ap()
    wk_t = nc.dram_tensor("wk_t", [D, CH], DT, kind="ExternalInput").ap()
    wv_t = nc.dram_tensor("wv_t", [D, D], DT, kind="ExternalInput").ap()
    wo_t = nc.dram_tensor("wo_t", [D, D], DT, kind="ExternalInput").ap()
    wsq_q = nc.dram_tensor("wsq_q", [128, CH], DT, kind="ExternalInput").ap()
    wsq_k = nc.dram_tensor("wsq_k", [128, CH], DT, kind="ExternalInput").ap()
    lam_in = nc.dram_tensor("lam", [1, 1], F32, kind="ExternalInput").ap()
    ones_one_in = nc.dram_tensor("ones_one", [128, H], DT, kind="ExternalInput").ap()
    if has_beta:
        gs_k_in = nc.dram_tensor("gs_k", [128, CHT], F32, kind="ExternalInput").ap()
        bq_in = nc.dram_tensor("bq", [128, CHT], F32, kind="ExternalInput").ap()
        bk_in = nc.dram_tensor("bk", [128, CHT], F32, kind="ExternalInput").ap()
    yT = nc.dram_tensor("yT", [D, TLOC], F32, kind="ExternalOutput").ap()

    with tile.TileContext(nc) as tc:
        with (
            tc.tile_pool(name="const", bufs=1) as const,
            tc.tile_pool(name="dram", bufs=1, space="DRAM") as dram,
            tc.tile_pool(name="rdd_pool", bufs=4, space="DRAM") as rdd_pool,
        ):
            # constants + tiny inputs
            ones_one = const.tile([128, H], DT)
            nc.sync.dma_start(ones_one[:], ones_one_in[:])
            lam_sb = const.tile([1, 1], F32)
            nc.sync.dma_start(lam_sb[:], lam_in[:])
            wsq_q_sb = const.tile([128, CH], DT)
            nc.sync.dma_start(wsq_q_sb[:], wsq_q[:])
            wsq_k_sb = const.tile([128, CH], DT)
            nc.sync.dma_start(wsq_k_sb[:], wsq_k[:])
            gsk_sb = bq_sb = bk_sb = None
            if has_beta:
                gsk_sb = const.tile([128, CHT], F32)
                nc.sync.dma_start(gsk_sb[:], gs_k_in[:])
                bq_sb = const.tile([128, CHT], F32)
                nc.sync.dma_start(bq_sb[:], bq_in[:])
                bk_sb = const.tile([128, CHT], F32)
                nc.sync.dma_start(bk_sb[:], bk_in[:])

            # DRAM bounce/gather buffers. kT_loc0 carries branch-0 K in
            # rows 0..CH/2-1 plus rstd_k as row CH/2 (rides the same gather).
            KR0 = CH // 2 + 1 if fast_k else CH // 2
            kT_loc0 = dram.tile([KR0, TLOC], DT)
            kT_loc1 = dram.tile([CH // 2, TLOC], DT)
            kT_g0 = dram.tile([GS, KR0, TLOC], DT)
            kT_g1 = dram.tile([GS, CH // 2, TLOC], DT)
            vA_loc = dram.tile([TLOC, VW], DT)
            vA_g = dram.tile([GS, TLOC, VW], DT)

            # ---------------- Phase 1: projections + LN + gathers -------
            with (
                tc.tile_pool(name="xp", bufs=1) as xp,
                tc.tile_pool(name="wv_p", bufs=KT) as wv_p,
                tc.tile_pool(name="wk_p", bufs=KT) as wk_p,
                tc.tile_pool(name="wq_p", bufs=KT) as wq_p,
                tc.tile_pool(name="prime_p", bufs=1) as prime_p,
                tc.tile_pool(name="sq_p", bufs=3) as sq_p,
                tc.tile_pool(name="stat_p", bufs=4) as stat_p,
                tc.tile_pool(name="ev_p", bufs=3) as ev_p,
                tc.tile_pool(name="qf_p", bufs=1) as qf_p,
                tc.tile_pool(name="vaug_p", bufs=2) as vaug_p,
                tc.tile_pool(name="pp", bufs=1, space="PSUM") as pp,
            ):
                xT_sb = xp.tile([128, KT, TLOC], DT)
                nc.sync.dma_start(xT_sb[:], xT.rearrange("(k p) t -> p k t", p=128))

                def load_strips(pool, dram_t, width, tag):
                    strips = []
                    for j in range(KT):
                        ws = pool.tile([128, width], DT, tag=tag, name=f"w_{tag}{j}")
                        nc.sync.dma_start(ws[:], dram_t[j * 128 : (j + 1) * 128, :])
                        strips.append(ws)
                    return strips

                # ---- K projection (first: its gather gates attention) ----
                kstrips = load_strips(wk_p, wk_t, CH, "wk")
                kprime = None
                if not fast_k:
                    kprime = prime_p.tile([128, CHT, TLOC], F32, tag="kprime")
                k_var = pp.tile([128, TLOC], F32, tag="var", bufs=2)
                for t in range(CHT):
                    ps = pp.tile([128, TLOC], F32, tag="proj", bufs=2)
                    for j in range(KT):
                        nc.tensor.matmul(
                            ps[:],
                            kstrips[j][:, t * 128 : (t + 1) * 128],
                            xT_sb[:, j, :],
                            start=(j == 0),
                            stop=(j == KT - 1),
                        )
                    sq = sq_p.tile([128, TLOC], DT, tag="sq")
                    nc.scalar.square(sq[:], ps[:])
                    nc.tensor.matmul(
                        k_var[:],
                        wsq_k_sb[:, t * 128 : (t + 1) * 128],
                        sq[:],
                        start=(t == 0),
                        stop=(t == CHT - 1),
                    )
                    if fast_k:
                        kf = ev_p.tile([128, TLOC], DT, tag="kf", name=f"kf{t}")
                        nc.vector.tensor_copy(kf[:], ps[:])
                        if t < CHT // 2:
                            nc.sync.dma_start(
                                kT_loc0[t * 128 : (t + 1) * 128, :], kf[:]
                            )
                        else:
                            tt = t - CHT // 2
                            nc.sync.dma_start(
                                kT_loc1[tt * 128 : (tt + 1) * 128, :], kf[:]
                            )
                    else:
                        nc.vector.tensor_copy(kprime[:, t, :], ps[:])

                if fast_k:
                    # rstd_k: slim [1, TLOC] chain; rides row CH/2 of kT_loc0
                    kv1 = stat_p.tile([1, TLOC], F32, tag="kv1")
                    nc.vector.tensor_scalar_add(kv1[:], k_var[0:1, :], EPS)
                    kv2 = stat_p.tile([1, TLOC], F32, tag="kv2")
                    nc.vector.reciprocal(kv2[:], kv1[:])
                    kv3 = stat_p.tile([1, TLOC], F32, tag="kv3")
                    nc.scalar.sqrt(kv3[:], kv2[:])
                    kv3d = stat_p.tile([1, TLOC], DT, tag="kv3d")
                    nc.vector.tensor_copy(kv3d[:], kv3[:])
                    nc.sync.dma_start(kT_loc0[CH // 2 : CH // 2 + 1, :], kv3d[:])
                else:
                    veps = stat_p.tile([128, TLOC], F32, tag="veps")
                    nc.vector.tensor_scalar_add(veps[:], k_var[:], EPS)
                    rec = stat_p.tile([128, TLOC], F32, tag="rec")
                    nc.vector.reciprocal(rec[:], veps[:])
                    rstd = stat_p.tile([128, TLOC], F32, tag="rstd")
                    nc.scalar.sqrt(rstd[:], rec[:])
                    for t in range(CHT):
                        kf = ev_p.tile([128, TLOC], DT, tag="kf", name=f"kfs{t}")
                        # gamma is already folded into the weights host-side
                        nc.vector.scalar_tensor_tensor(
                            out=kf[:],
                            in0=kprime[:, t, :],
                            scalar=1.0,
                            in1=rstd[:],
                            op0=mybir.AluOpType.mult,
                            op1=mybir.AluOpType.mult,
                        )
                        nc.vector.tensor_scalar_add(kf[:], kf[:], bk_sb[:, t : t + 1])
                        if t < CHT // 2:
                            nc.sync.dma_start(
                                kT_loc0[t * 128 : (t + 1) * 128, :], kf[:]
                            )
                        else:
                            tt = t - CHT // 2
                            nc.sync.dma_start(
                                kT_loc1[tt * 128 : (tt + 1) * 128, :], kf[:]
                            )

                # gathers: k0(+rstd), v, k1 — in cc-queue order
                nc.gpsimd.collective_compute(
                    "AllGather",
                    mybir.AluOpType.bypass,
                    replica_groups=GROUPS,
                    ins=[kT_loc0[:]],
                    outs=[kT_g0[:]],
                )

                # ---- V projection ----
                vstrips = load_strips(wv_p, wv_t, D, "wv")
                for c in range(TC):
                    vps = pp.tile([128, D], F32, tag="vproj", bufs=2)
                    for j in range(KT):
                        for n in range(D // 512):
                            nc.tensor.matmul(
                                vps[:, n * 512 : (n + 1) * 512],
                                xT_sb[:, j, c * 128 : (c + 1) * 128],
                                vstrips[j][:, n * 512 : (n + 1) * 512],
                                start=(j == 0),
                                stop=(j == KT - 1),
                            )
                    vaug = vaug_p.tile([128, H, 65], DT, tag="vaug")
                    nc.vector.tensor_copy(
                        vaug[:, :, 0:64], vps[:].rearrange("p (h d) -> p h d", h=H)
                    )
                    nc.sync.dma_start(vaug[:, :, 64:65], ones_one[:])
                    nc.sync.dma_start(
                        vA_loc[c * 128 : (c + 1) * 128, :],
                        vaug.rearrange("p h d -> p (h d)"),
                    )
                nc.gpsimd.collective_compute(
                    "AllGather",
                    mybir.AluOpType.bypass,
                    replica_groups=GROUPS,
                    ins=[vA_loc[:]],
                    outs=[vA_g[:]],
                )
                nc.gpsimd.collective_compute(
                    "AllGather",
                    mybir.AluOpType.bypass,
                    replica_groups=GROUPS,
                    ins=[kT_loc1[:]],
                    outs=[kT_g1[:]],
                )

                # ---- Q projection (stays resident; LN applied on device) ----
                qstrips = load_strips(wq_p, wq_t, CH, "wq")
                qf_sb = qf_p.tile([128, CHT, TLOC], DT)
                qprime = prime_p.tile([128, CHT, TLOC], F32, tag="qprime")
                q_var = pp.tile([128, TLOC], F32, tag="var", bufs=2)
                for t in range(CHT):
                    ps = pp.tile([128, TLOC], F32, tag="proj", bufs=2)
                    for j in range(KT):
                        nc.tensor.matmul(
                            ps[:],
                            qstrips[j][:, t * 128 : (t + 1) * 128],
                            xT_sb[:, j, :],
                            start=(j == 0),
                            stop=(j == KT - 1),
                        )
                    nc.vector.tensor_copy(qprime[:, t, :], ps[:])
                    sq = sq_p.tile([128, TLOC], DT, tag="sq")
                    nc.scalar.square(sq[:], ps[:])
                    nc.tensor.matmul(
                        q_var[:],
                        wsq_q_sb[:, t * 128 : (t + 1) * 128],
                        sq[:],
                        start=(t == 0),
                        stop=(t == CHT - 1),
                    )
                veps = stat_p.tile([128, TLOC], F32, tag="veps")
                nc.vector.tensor_scalar_add(veps[:], q_var[:], EPS)
                rec = stat_p.tile([128, TLOC], F32, tag="rec")
                nc.vector.reciprocal(rec[:], veps[:])
                rstd = stat_p.tile([128, TLOC], F32, tag="rstd")
                nc.scalar.sqrt(rstd[:], rec[:])
                for t in range(CHT):
                    nc.vector.scalar_tensor_tensor(
                        out=qf_sb[:, t, :],
                        in0=qprime[:, t, :],
                        scalar=DH**-0.5,
                        in1=rstd[:],
                        op0=mybir.AluOpType.mult,
                        op1=mybir.AluOpType.mult,
                    )
                    if has_beta:
                        nc.vector.tensor_scalar_add(
                            qf_sb[:, t, :], qf_sb[:, t, :], bq_sb[:, t : t + 1]
                        )

            # ---------------- Phase 2: attention ------------------------
            with (
                tc.tile_pool(name="rk_p", bufs=1) as rk_p,
                tc.tile_pool(name="kh_p", bufs=4) as kh_p,
                tc.tile_pool(name="vh_p", bufs=3) as vh_p,
                tc.tile_pool(name="pt_p", bufs=NKC + 2) as pt_p,
                tc.tile_pool(name="o1_p", bufs=HP) as o1_p,
                tc.tile_pool(name="rd_p", bufs=4) as rd_p,
                tc.tile_pool(name="rdb_p", bufs=4) as rdb_p,
                tc.tile_pool(name="attn_p", bufs=1) as attn_p,
                tc.tile_pool(name="scp", bufs=3, space="PSUM") as scp,
                tc.tile_pool(name="avp", bufs=2, space="PSUM") as avp,
            ):
                attn_sb = attn_p.tile([128, HP, TLOC], DT)
                if fast_k:
                    rk_dt = rk_p.tile([128, NKC], DT)
                    for g in range(GS):
                        nc.sync.dma_start(
                            rk_dt[:, g * L : (g + 1) * L],
                            kT_g0[g, CH // 2, :].rearrange("(l p) -> p l", p=128),
                        )
                    rk_sb = rk_p.tile([128, NKC], F32)
                    nc.vector.tensor_copy(rk_sb[:], rk_dt[:])
                o1_tiles = {}
                for b in range(2):
                    kT_gb = kT_g0 if b == 0 else kT_g1
                    for hp in range(HP):
                        vh = vh_p.tile([128, NKC, 130], DT, tag="vh")
                        for g in range(GS):
                            nc.sync.dma_start(
                                vh[:, g * L : (g + 1) * L, :],
                                vA_g[
                                    g, :, (2 * hp) * 65 : (2 * hp + 2) * 65
                                ].rearrange("(l p) c -> p l c", p=128),
                            )
                        kh = kh_p.tile([128, t_total], DT, tag="kh")
                        for g in range(GS):
                            nc.sync.dma_start(
                                kh[:, g * TLOC : (g + 1) * TLOC],
                                kT_gb[g, hp * 128 : (hp + 1) * 128, :],
                            )
                        if b == 0:
                            o1 = o1_p.tile([128, TLOC], F32, tag="o1", name=f"o1_{hp}")
                            o1_tiles[hp] = o1
                        else:
                            o1 = o1_tiles[hp]
                        avE = avp.tile([65, TLOC], F32, tag="av")
                        avO = avp.tile([65, TLOC], F32, tag="av")
                        qE = qf_sb[0:64, b * (CHT // 2) + hp, :]
                        qO = qf_sb[64:128, b * (CHT // 2) + hp, :]
                        # phase A: all scores + exps (PE never waits on ACT)
                        pts = []
                        for c in range(NKC):
                            sc = scp.tile([128, 2, TLOC], F32, tag="sc")
                            nc.tensor.matmul(
                                sc[:, 0, :],
                                kh[0:64, c * 128 : (c + 1) * 128],
                                qE,
                                start=True,
                                stop=True,
                            )
                            nc.tensor.matmul(
                                sc[:, 1, :],
                                kh[64:128, c * 128 : (c + 1) * 128],
                                qO,
                                start=True,
                                stop=True,
                            )
                            pt = pt_p.tile([128, 2, TLOC], DT, tag="pt")
                            if fast_k:
                                nc.scalar.activation(
                                    pt[:],
                                    sc[:],
                                    mybir.ActivationFunctionType.Exp,
                                    scale=rk_sb[:, c : c + 1],
                                )
                            else:
                                nc.scalar.activation(
                                    pt[:], sc[:], mybir.ActivationFunctionType.Exp
                                )
                            pts.append(pt)
                        # phase B: all AV matmuls
                        for c in range(NKC):
                            nc.tensor.matmul(
                                avE[:],
                                vh[:, c, 0:65],
                                pts[c][:, 0, :],
                                start=(c == 0),
                                stop=(c == NKC - 1),
                            )
                            nc.tensor.matmul(
                                avO[:],
                                vh[:, c, 65:130],
                                pts[c][:, 1, :],
                                start=(c == 0),
                                stop=(c == NKC - 1),
                            )
                        for parity, av in ((0, avE), (1, avO)):
                            # denominator -> reciprocal (rebased to partition 0)
                            rdc = rd_p.tile([1, TLOC], F32, tag="rdc")
                            nc.vector.tensor_copy(rdc[:], av[64:65, :])
                            rd = rd_p.tile([1, TLOC], F32, tag="rd")
                            nc.vector.reciprocal(rd[:], rdc[:])
                            if b == 1:
                                nc.vector.tensor_scalar_mul(
                                    rd[:], rd[:], lam_sb[0:1, 0:1]
                                )
                            rdd = rdd_pool.tile([1, TLOC], F32, tag="rdd")
                            nc.sync.dma_start(rdd[:], rd[:])
                            rows = slice(parity * 64, parity * 64 + 64)
                            rdb = rdb_p.tile([128, TLOC], F32, tag="rdb")
                            nc.sync.dma_start(rdb[rows, :], _bcast_part(rdd[:], 64))
                            if b == 0:
                                nc.vector.tensor_mul(
                                    o1[rows, :], av[0:64, :], rdb[rows, :]
                                )
                            else:
                                o2 = rdb_p.tile([128, TLOC], F32, tag="o2")
                                nc.vector.tensor_mul(
                                    o2[rows, :], av[0:64, :], rdb[rows, :]
                                )
                                nc.vector.tensor_sub(
                                    attn_sb[rows, hp, :], o1[rows, :], o2[rows, :]
                                )

            # ---------------- Phase 3: output projection ----------------
            with (
                tc.tile_pool(name="wo_p", bufs=KT) as wo_p,
                tc.tile_pool(name="ye_p", bufs=3) as ye_p,
                tc.tile_pool(name="yp", bufs=2, space="PSUM") as yp,
            ):
                wostrips = []
                for j in range(KT):
                    ws = wo_p.tile([128, D], DT, tag="wo", name=f"w_o{j}")
                    nc.sync.dma_start(ws[:], wo_t[j * 128 : (j + 1) * 128, :])
                    wostrips.append(ws)
                for dt_ in range(D // 128):
                    yps = yp.tile([128, TLOC], F32, tag="y")
                    for j in range(KT):
                        nc.tensor.matmul(
                            yps[:],
                            wostrips[j][:, dt_ * 128 : (dt_ + 1) * 128],
                            attn_sb[:, j, :],
                            start=(j == 0),
                            stop=(j == KT - 1),
                        )
                    ye = ye_p.tile([128, TLOC], F32, tag="ye")
                    nc.vector.tensor_copy(ye[:], yps[:])
                    nc.sync.dma_start(yT[dt_ * 128 : (dt_ + 1) * 128, :], ye[:])

    nc.compile()
    return nc


# ---------------- host-side preparation ----------------


def _quantize(W):
    W = np.asarray(W, dtype=np.float32)
    scale = np.clip(np.abs(W).mean(axis=1, keepdims=True), 1e-5, None)
    wq = np.clip(np.round(W / scale), -1.0, 1.0)
    return (wq * scale).astype(np.float32)


def prepare_inputs(
    x, Wq, Wk, Wv, Wo, lambda_q, lambda_k, qn_gamma, qn_beta, kn_gamma, kn_beta,
    mm_dt=MM_DT,
):
    """Host prep: quantize + center weights, fold gamma, per-core slices."""
    np_dt = mybir.dt.np(_DT_MAP[mm_dt])
    x = np.asarray(x, dtype=np.float32)
    t_total = x.shape[1]
    ch = 2 * H * DH
    cht = ch // 128
    tloc = t_total // GS

    Wq_e = _quantize(Wq)
    Wk_e = _quantize(Wk)
    Wv_e = _quantize(Wv)
    Wo_e = _quantize(Wo)
    # fold LN mean-subtraction into column-centered weights, gamma into rows
    gq = np.asarray(qn_gamma, np.float32)
    gk = np.asarray(kn_gamma, np.float32)
    Wq_c = (Wq_e - Wq_e.mean(axis=0, keepdims=True)) * gq[:, None]
    Wk_c = (Wk_e - Wk_e.mean(axis=0, keepdims=True)) * gk[:, None]

    wq_t = np.ascontiguousarray(Wq_c.T).astype(np_dt)
    wk_t = np.ascontiguousarray(Wk_c.T).astype(np_dt)
    wv_t = np.ascontiguousarray(Wv_e.T).astype(np_dt)
    wo_t = np.ascontiguousarray(Wo_e.T).astype(np_dt)

    def wsq_full(g):
        # stationary for the variance matmul: [128, cht*128] where column
        # block t is constant per-partition 1/(ch * gamma[t*128+p]^2)
        w = 1.0 / (ch * np.maximum(g, 1e-12) ** 2)
        return np.ascontiguousarray(
            np.repeat(w.reshape(cht, 128).T[:, :, None], 128, axis=2).reshape(
                128, cht * 128
            )
        ).astype(np_dt)

    lam = np.clip(
        np.exp(np.asarray(lambda_q).mean() - np.asarray(lambda_k).mean()), 0.1, 2.0
    ).astype(np.float32)

    has_beta = bool(np.any(np.asarray(qn_beta)) or np.any(np.asarray(kn_beta)))
    common = {
        "wq_t": wq_t,
        "wk_t": wk_t,
        "wv_t": wv_t,
        "wo_t": wo_t,
        "wsq_q": wsq_full(gq),
        "wsq_k": wsq_full(gk),
        "lam": lam.reshape(1, 1),
        "ones_one": np.ones((128, H), np_dt),
    }
    if has_beta:
        bq = np.asarray(qn_beta, np.float32) * (DH**-0.5)
        bk = np.asarray(kn_beta, np.float32)
        common["gs_k"] = np.ascontiguousarray(gk.reshape(cht, 128).T)
        common["bq"] = np.ascontiguousarray(bq.reshape(cht, 128).T)
        common["bk"] = np.ascontiguousarray(bk.reshape(cht, 128).T)

    in_maps = []
    for c in range(NCORES):
        b = c // GS
        ts = c % GS
        xt = np.ascontiguousarray(x[b, ts * tloc : (ts + 1) * tloc, :].T).astype(np_dt)
        in_maps.append({**common, "xT": xt})
    return in_maps, has_beta, t_total


def get_program(t_total=T, has_beta=False, mm_dt=MM_DT):
    key = (t_total, has_beta, mm_dt)
    if key not in _PROG_CACHE:
        _PROG_CACHE[key] = build_program(t_total, has_beta, mm_dt)
    return _PROG_CACHE[key]


def run(inputs, trace=False, mm_dt=MM_DT):
    """Run on hardware; returns (full_output, BassKernelResults)."""
    in_maps, has_beta, t_total = prepare_inputs(**inputs, mm_dt=mm_dt)
    nc = get_program(t_total, has_beta, mm_dt)
    res = run_bass_kernel_spmd(nc, in_maps, list(range(NCORES)), trace=trace)
    tloc = t_total // GS
    out = np.empty((B, t_total, D), dtype=np.float32)
    for c in range(NCORES):
        b = c // GS
        ts = c % GS
        out[b, ts * tloc : (ts + 1) * tloc, :] = res.results[c]["yT"].T
    return out, res


def kernel(**inputs) -> np.ndarray:
    out, _ = run(inputs, trace=False)
    return out


# revision 11
# speedup vs baseline: 1.6094x; 1.0154x over previous
"""DifferentialAttention Trainium2 kernel (8 NeuronCores, SPMD).

Sharding: 2 batches x 4 token-blocks = 8 cores. Each core computes
projections + layernorm for its 512 tokens, AllGathers K/V within its
4-core (same-batch) group, then computes attention + output projection
for its own query tokens. Host stitches per-core [D, 512] outputs.

Layout strategy: everything "transposed" (features on partitions, tokens
on free dim) so the whole chain q-proj -> scores -> AV -> out-proj needs
zero on-device transposes:
  - LN mean subtraction is folded into host-side column-centering of the
    (ternary-quantized) weight matrices, and gamma is folded into the
    weight rows; only the variance needs device work (a matmul of the
    squared activations against a per-channel 1/(CH*gamma^2) stationary).
  - K's 1/std never touches K on device: it rides as the per-partition
    `scale` operand of the softmax exp (scoresT has keys on partitions),
    so raw centered K can be gathered as soon as its projection finishes.
  - softmax runs without max-subtraction (scores are O(+-8); exp is safe)
    and the denominator comes for free as a 65th "ones" column appended
    to V.
  - lambda (a host-computable scalar) rides in as a [1,1] input tensor so
    the compiled program is input-independent.

Attention is branch-outer, and each (branch, head-pair) runs all 16
score-matmuls + exps first, then all 32 AV matmuls — the PE's in-order
queue never waits on the ScalarE exp stream.

MM_DT picks the matmul operand dtype: float16 (default), bfloat16, or
float32r.
"""

import os
import sys
import types

for _p in ("/opt/trn_rl_repo",):
    if os.path.isdir(_p) and _p not in sys.path:
        sys.path.append(_p)

import numpy as np

import concourse.bass as bass
import concourse.tile as tile
from concourse.bass import _add_dep_helper
from concourse import bacc, mybir
from concourse.bass_utils import run_bass_kernel_spmd


def _install_ntff_shim():
    """bass_utils imports antenv.axon_hooks when tracing under axon; the
    container antenv stub lacks it. Back it with the ctypes hook."""
    if "antenv.axon_hooks" in sys.modules:
        return
    try:
        from trn_agent_boot.trn_boot import _ntff_profile_via_ctypes

        hook = _ntff_profile_via_ctypes("/opt/axon/libaxon_pjrt.so")
    except Exception:
        hook = None
    mod = types.ModuleType("antenv.axon_hooks")
    mod.get_axon_ntff_profile_hook = lambda: hook
    sys.modules["antenv.axon_hooks"] = mod


_install_ntff_shim()

# ----- problem dims (hardcoded per spec) -----
B, T, D = 2, 2048, 1024
H, DH = 16, 64
CH = 2 * H * DH  # 2048
EPS = 1e-5
NCORES = 8
GS = 4  # cores per batch group
GROUPS = [[0, 1, 2, 3], [4, 5, 6, 7]]

F32 = mybir.dt.float32
MM_DT = "f16"  # "f16" | "bf16" | "f32r"
_DT_MAP = {
    "f16": mybir.dt.float16,
    "bf16": mybir.dt.bfloat16,
    "f32r": mybir.dt.float32r,
}

_PROG_CACHE: dict = {}


def _bcast_part(ap, n):
    """AP view replicating a 1-partition AP across n partitions (step 0)."""
    return bass.AP(tensor=ap.tensor, offset=ap.offset, ap=[[0, n]] + list(ap.ap)[1:])


def build_program(t_total=T, has_beta=False, mm_dt=MM_DT):
    """Build the per-core SPMD program. t_total is the per-batch sequence
    length (2048 for the real problem; smaller for simulation)."""
    TLOC = t_total // GS  # query tokens owned by this core
    KT = D // 128  # contraction tiles for projections
    CHT = CH // 128  # channel tiles of q/k
    NKC = t_total // 128  # key chunks
    TC = TLOC // 128  # token chunks (v projection)
    L = TLOC // 128  # 128-blocks per gather block
    HP = H // 2  # head pairs
    VW = H * 65  # augmented v width (64 + ones column per head)
    DT = _DT_MAP[mm_dt]
    fast_k = not has_beta  # beta on K forces the on-device LN-apply path

    nc = bacc.Bacc("TRN2", target_bir_lowering=False, debug=False, num_devices=NCORES)

    xT = nc.dram_tensor("xT", [D, TLOC], DT, kind="ExternalInput").ap()
    wq_t = nc.dram_tensor("wq_t", [D, CH], DT, kind="ExternalInput").ap()
    wk_t = nc.dram_tensor("wk_t", [D, CH], DT, kind="ExternalInput").ap()
    wv_t = nc.dram_tensor("wv_t", [D, D], DT, kind="ExternalInput").ap()
    wo_t = nc.dram_tensor("wo_t", [D, D], DT, kind="ExternalInput").ap()
    wsq_q = nc.dram_tensor("wsq_q", [128, CH], DT, kind="ExternalInput").ap()
    wsq_k = nc.dram_tensor("wsq_k", [128, CH], DT, kind="ExternalInput").ap()
    lam_in = nc.dram_tensor("lam", [1, 1], F32, kind="ExternalInput").ap()
    ones_one_in = nc.dram_tensor("ones_one", [128, H], DT, kind="ExternalInput").ap()
    if has_beta:
        gs_k_in = nc.dram_tensor("gs_k", [128, CHT], F32, kind="ExternalInput").ap()
        bq_in = nc.dram_tensor("bq", [128, CHT], F32, kind="ExternalInput").ap()
        bk_in = nc.dram_tensor("bk", [128, CHT], F32, kind="ExternalInput").ap()
    yT = nc.dram_tensor("yT", [D, TLOC], F32, kind="ExternalOutput").ap()

    with tile.TileContext(nc) as tc:
        with (
            tc.tile_pool(name="const", bufs=1) as const,
            tc.tile_pool(name="dram", bufs=1, space="DRAM") as dram,
            tc.tile_pool(name="rdd_pool", bufs=4, space="DRAM") as rdd_pool,
            tc.tile_pool(name="qf_p", bufs=1) as qf_p,
            tc.tile_pool(name="attn_p", bufs=1) as attn_p,
        ):
            # constants + tiny inputs
            ones_one = const.tile([128, H], DT)
            nc.sync.dma_start(ones_one[:], ones_one_in[:])
            lam_sb = const.tile([1, 1], F32)
            nc.sync.dma_start(lam_sb[:], lam_in[:])
            wsq_q_sb = const.tile([128, CH], DT)
            nc.sync.dma_start(wsq_q_sb[:], wsq_q[:])
            wsq_k_sb = const.tile([128, CH], DT)
            nc.sync.dma_start(wsq_k_sb[:], wsq_k[:])
            gsk_sb = bq_sb = bk_sb = None
            if has_beta:
                gsk_sb = const.tile([128, CHT], F32)
                nc.sync.dma_start(gsk_sb[:], gs_k_in[:])
                bq_sb = const.tile([128, CHT], F32)
                nc.sync.dma_start(bq_sb[:], bq_in[:])
                bk_sb = const.tile([128, CHT], F32)
                nc.sync.dma_start(bk_sb[:], bk_in[:])

            # DRAM bounce/gather buffers. kT_loc0 carries branch-0 K in
            # rows 0..CH/2-1 plus rstd_k as row CH/2 (rides the same gather).
            KR0 = CH // 2 + 2 if fast_k else CH // 2
            kT_loc0 = dram.tile([KR0, TLOC], DT)
            kT_loc1 = dram.tile([CH // 2, TLOC], DT)
            kT_g0 = dram.tile([GS, KR0, TLOC], DT)
            kT_g1 = dram.tile([GS, CH // 2, TLOC], DT)
            vA_loc = dram.tile([TLOC, VW], DT)
            vA_g = dram.tile([GS, TLOC, VW], DT)

            # ---------------- Phase 1: projections + LN + gathers -------
            with (
                tc.tile_pool(name="xp", bufs=1) as xp,
                tc.tile_pool(name="w_p", bufs=KT) as w_p,
                tc.tile_pool(name="prime_p", bufs=1) as prime_p,
                tc.tile_pool(name="sq_p", bufs=3) as sq_p,
                tc.tile_pool(name="stat_p", bufs=2) as stat_p,
                tc.tile_pool(name="ev_p", bufs=3) as ev_p,
                tc.tile_pool(name="vaug_p", bufs=2) as vaug_p,
                tc.tile_pool(name="pp", bufs=1, space="PSUM") as pp,
            ):
                xT_sb = xp.tile([128, KT, TLOC], DT)
                nc.sync.dma_start(xT_sb[:], xT.rearrange("(k p) t -> p k t", p=128))

                def load_strips(pool, dram_t, width, tag):
                    # all strips share one slot-cycled pool sized [128, width]
                    strips = []
                    dw = dram_t.shape[1]
                    for j in range(KT):
                        ws = pool.tile([128, width], DT, tag=tag, name=f"w_{tag}{j}")
                        nc.sync.dma_start(
                            ws[:, 0:dw], dram_t[j * 128 : (j + 1) * 128, :]
                        )
                        strips.append(ws)
                    return strips

                # ---- K projection (first: its gather gates attention) ----
                kstrips = load_strips(w_p, wk_t, CH, "w")
                kprime = None
                if not fast_k:
                    kprime = prime_p.tile([128, CHT, TLOC], F32, tag="kprime")
                k_var = pp.tile([128, TLOC], F32, tag="var", bufs=2)
                for t in range(CHT):
                    ps = pp.tile([128, TLOC], F32, tag="proj", bufs=2)
                    for j in range(KT):
                        nc.tensor.matmul(
                            ps[:],
                            kstrips[j][:, t * 128 : (t + 1) * 128],
                            xT_sb[:, j, :],
                            start=(j == 0),
                            stop=(j == KT - 1),
                        )
                    sq = sq_p.tile([128, TLOC], DT, tag="sq")
                    if fast_k:
                        kf = ev_p.tile([128, TLOC], DT, tag="kf", name=f"kf{t}")
                        nc.vector.tensor_copy(kf[:], ps[:])
                        nc.scalar.square(sq[:], ps[:])
                        if t < CHT // 2:
                            nc.sync.dma_start(
                                kT_loc0[t * 128 : (t + 1) * 128, :], kf[:]
                            )
                        else:
                            tt = t - CHT // 2
                            nc.sync.dma_start(
                                kT_loc1[tt * 128 : (tt + 1) * 128, :], kf[:]
                            )
                    else:
                        nc.vector.tensor_copy(kprime[:, t, :], ps[:])
                        nc.scalar.square(sq[:], ps[:])
                    nc.tensor.matmul(
                        k_var[:],
                        wsq_k_sb[:, t * 128 : (t + 1) * 128],
                        sq[:],
                        start=(t == 0),
                        stop=(t == CHT - 1),
                    )

                if fast_k:
                    # rstd_k: slim [1, TLOC] chain; rides row CH/2 of kT_loc0
                    kv1 = stat_p.tile([1, TLOC], F32, tag="kv1")
                    nc.vector.tensor_scalar_add(kv1[:], k_var[0:1, :], EPS)
                    kv2 = stat_p.tile([1, TLOC], F32, tag="kv2")
                    nc.vector.reciprocal(kv2[:], kv1[:])
                    kv3 = stat_p.tile([1, TLOC], F32, tag="kv3")
                    nc.scalar.sqrt(kv3[:], kv2[:])
                    # split into hi+lo fp16 parts (sum reconstructs ~21
                    # mantissa bits; a bare fp16 rstd perturbs logits by up to
                    # |score| * 2^-11 which is too coarse)
                    kv3d = stat_p.tile([1, TLOC], DT, tag="kv3d")
                    nc.vector.tensor_copy(kv3d[:], kv3[:])
                    kv3dF = stat_p.tile([1, TLOC], F32, tag="kv3dF")
                    nc.vector.tensor_copy(kv3dF[:], kv3d[:])
                    kv3lo = stat_p.tile([1, TLOC], DT, tag="kv3lo")
                    nc.vector.tensor_sub(kv3lo[:], kv3[:], kv3dF[:])
                    nc.sync.dma_start(kT_loc0[CH // 2 : CH // 2 + 1, :], kv3d[:])
                    nc.sync.dma_start(kT_loc0[CH // 2 + 1 : CH // 2 + 2, :], kv3lo[:])
                else:
                    veps = stat_p.tile([128, TLOC], F32, tag="veps")
                    nc.vector.tensor_scalar_add(veps[:], k_var[:], EPS)
                    rec = stat_p.tile([128, TLOC], F32, tag="rec")
                    nc.vector.reciprocal(rec[:], veps[:])
                    rstd = stat_p.tile([128, TLOC], F32, tag="rstd")
                    nc.scalar.sqrt(rstd[:], rec[:])
                    for t in range(CHT):
                        kf = ev_p.tile([128, TLOC], DT, tag="kf", name=f"kfs{t}")
                        # gamma is already folded into the weights host-side
                        nc.vector.scalar_tensor_tensor(
                            out=kf[:],
                            in0=kprime[:, t, :],
                            scalar=1.0,
                            in1=rstd[:],
                            op0=mybir.AluOpType.mult,
                            op1=mybir.AluOpType.mult,
                        )
                        nc.vector.tensor_scalar_add(kf[:], kf[:], bk_sb[:, t : t + 1])
                        if t < CHT // 2:
                            nc.sync.dma_start(
                                kT_loc0[t * 128 : (t + 1) * 128, :], kf[:]
                            )
                        else:
                            tt = t - CHT // 2
                            nc.sync.dma_start(
                                kT_loc1[tt * 128 : (tt + 1) * 128, :], kf[:]
                            )

                # gathers: k0(+rstd), v, k1 — explicitly chained so the
                # scheduler cannot reorder them on the cc queue
                cc_k0 = nc.gpsimd.collective_compute(
                    "AllGather",
                    mybir.AluOpType.bypass,
                    replica_groups=GROUPS,
                    ins=[kT_loc0[:]],
                    outs=[kT_g0[:]],
                )

                # ---- V projection ----
                vstrips = load_strips(w_p, wv_t, CH, "w")
                for c in range(TC):
                    vps = pp.tile([128, D], F32, tag="vproj", bufs=2)
                    for j in range(KT):
                        for n in range(D // 512):
                            nc.tensor.matmul(
                                vps[:, n * 512 : (n + 1) * 512],
                                xT_sb[:, j, c * 128 : (c + 1) * 128],
                                vstrips[j][:, n * 512 : (n + 1) * 512],
                                start=(j == 0),
                                stop=(j == KT - 1),
                            )
                    vaug = vaug_p.tile([128, H, 65], DT, tag="vaug")
                    nc.vector.tensor_copy(
                        vaug[:, :, 0:64], vps[:].rearrange("p (h d) -> p h d", h=H)
                    )
                    nc.sync.dma_start(vaug[:, :, 64:65], ones_one[:])
                    nc.sync.dma_start(
                        vA_loc[c * 128 : (c + 1) * 128, :],
                        vaug.rearrange("p h d -> p (h d)"),
                    )
                cc_v = nc.gpsimd.collective_compute(
                    "AllGather",
                    mybir.AluOpType.bypass,
                    replica_groups=GROUPS,
                    ins=[vA_loc[:]],
                    outs=[vA_g[:]],
                )
                _add_dep_helper(cc_v.ins, cc_k0.ins, sync=True, reason="cc order")
                cc_k1 = nc.gpsimd.collective_compute(
                    "AllGather",
                    mybir.AluOpType.bypass,
                    replica_groups=GROUPS,
                    ins=[kT_loc1[:]],
                    outs=[kT_g1[:]],
                )
                _add_dep_helper(cc_k1.ins, cc_v.ins, sync=True, reason="cc order")

                # ---- Q projection (stays resident; LN applied on device) ----
                qstrips = load_strips(w_p, wq_t, CH, "w")
                qf_sb = qf_p.tile([128, CHT, TLOC], DT)
                qprime = prime_p.tile([128, CHT, TLOC], F32, tag="qprime")
                q_var = pp.tile([128, TLOC], F32, tag="var", bufs=2)
                for t in range(CHT):
                    ps = pp.tile([128, TLOC], F32, tag="proj", bufs=2)
                    for j in range(KT):
                        nc.tensor.matmul(
                            ps[:],
                            qstrips[j][:, t * 128 : (t + 1) * 128],
                            xT_sb[:, j, :],
                            start=(j == 0),
                            stop=(j == KT - 1),
                        )
                    nc.vector.tensor_copy(qprime[:, t, :], ps[:])
                    sq = sq_p.tile([128, TLOC], DT, tag="sq")
                    nc.scalar.square(sq[:], ps[:])
                    nc.tensor.matmul(
                        q_var[:],
                        wsq_q_sb[:, t * 128 : (t + 1) * 128],
                        sq[:],
                        start=(t == 0),
                        stop=(t == CHT - 1),
                    )
                veps = stat_p.tile([128, TLOC], F32, tag="veps")
                nc.vector.tensor_scalar_add(veps[:], q_var[:], EPS)
                rec = stat_p.tile([128, TLOC], F32, tag="rec")
                nc.vector.reciprocal(rec[:], veps[:])
                rstd = stat_p.tile([128, TLOC], F32, tag="rstd")
                nc.scalar.sqrt(rstd[:], rec[:])
                for t in range(CHT):
                    nc.vector.scalar_tensor_tensor(
                        out=qf_sb[:, t, :],
                        in0=qprime[:, t, :],
                        scalar=DH**-0.5,
                        in1=rstd[:],
                        op0=mybir.AluOpType.mult,
                        op1=mybir.AluOpType.mult,
                    )
                    if has_beta:
                        nc.vector.tensor_scalar_add(
                            qf_sb[:, t, :], qf_sb[:, t, :], bq_sb[:, t : t + 1]
                        )

            # ---------------- Phase 2: attention ------------------------
            with (
                tc.tile_pool(name="rk_p", bufs=1) as rk_p,
                tc.tile_pool(name="kh_p", bufs=4) as kh_p,
                tc.tile_pool(name="vh_p", bufs=3) as vh_p,
                tc.tile_pool(name="pt_p", bufs=NKC + 2) as pt_p,
                tc.tile_pool(name="o1_p", bufs=HP) as o1_p,
                tc.tile_pool(name="rd_p", bufs=4) as rd_p,
                tc.tile_pool(name="rdb_p", bufs=4) as rdb_p,
                tc.tile_pool(name="scp", bufs=3, space="PSUM") as scp,
                tc.tile_pool(name="avp", bufs=2, space="PSUM") as avp,
            ):
                attn_sb = attn_p.tile([128, HP, TLOC], DT)
                if fast_k:
                    rk_hi = rk_p.tile([128, NKC], DT)
                    rk_lo = rk_p.tile([128, NKC], DT)
                    for g in range(GS):
                        nc.sync.dma_start(
                            rk_hi[:, g * L : (g + 1) * L],
                            kT_g0[g, CH // 2, :].rearrange("(l p) -> p l", p=128),
                        )
                        nc.sync.dma_start(
                            rk_lo[:, g * L : (g + 1) * L],
                            kT_g0[g, CH // 2 + 1, :].rearrange("(l p) -> p l", p=128),
                        )
                    rk_hiF = rk_p.tile([128, NKC], F32)
                    nc.vector.tensor_copy(rk_hiF[:], rk_hi[:])
                    rk_loF = rk_p.tile([128, NKC], F32)
                    nc.vector.tensor_copy(rk_loF[:], rk_lo[:])
                    rk_sb = rk_p.tile([128, NKC], F32)
                    nc.vector.tensor_add(rk_sb[:], rk_hiF[:], rk_loF[:])
                o1_tiles = {}
                for b in range(2):
                    kT_gb = kT_g0 if b == 0 else kT_g1
                    for hp in range(HP):
                        vh = vh_p.tile([128, NKC, 130], DT, tag="vh")
                        for g in range(GS):
                            nc.sync.dma_start(
                                vh[:, g * L : (g + 1) * L, :],
                                vA_g[
                                    g, :, (2 * hp) * 65 : (2 * hp + 2) * 65
                                ].rearrange("(l p) c -> p l c", p=128),
                            )
                        kh = kh_p.tile([128, t_total], DT, tag="kh")
                        for g in range(GS):
                            nc.sync.dma_start(
                                kh[:, g * TLOC : (g + 1) * TLOC],
                                kT_gb[g, hp * 128 : (hp + 1) * 128, :],
                            )
                        if b == 0:
                            o1 = o1_p.tile([128, TLOC], F32, tag="o1", name=f"o1_{hp}")
                            o1_tiles[hp] = o1
                        else:
                            o1 = o1_tiles[hp]
                        avE = avp.tile([65, TLOC], F32, tag="av")
                        avO = avp.tile([65, TLOC], F32, tag="av")
                        qE = qf_sb[0:64, b * (CHT // 2) + hp, :]
                        qO = qf_sb[64:128, b * (CHT // 2) + hp, :]
                        # phase A: all scores + exps (PE never waits on ACT)
                        pts = []
                        for c in range(NKC):
                            sc = scp.tile([128, 2, TLOC], F32, tag="sc")
                            nc.tensor.matmul(
                                sc[:, 0, :],
                                kh[0:64, c * 128 : (c + 1) * 128],
                                qE,
                                start=True,
                                stop=True,
                            )
                            nc.tensor.matmul(
                                sc[:, 1, :],
                                kh[64:128, c * 128 : (c + 1) * 128],
                                qO,
                                start=True,
                                stop=True,
                            )
                            pt = pt_p.tile([128, 2, TLOC], DT, tag="pt")
                            if fast_k:
                                nc.scalar.activation(
                                    pt[:],
                                    sc[:],
                                    mybir.ActivationFunctionType.Exp,
                                    scale=rk_sb[:, c : c + 1],
                                )
                            else:
                                nc.scalar.activation(
                                    pt[:], sc[:], mybir.ActivationFunctionType.Exp
                                )
                            pts.append(pt)
                        # phase B: all AV matmuls
                        for c in range(NKC):
                            nc.tensor.matmul(
                                avE[:],
                                vh[:, c, 0:65],
                                pts[c][:, 0, :],
                                start=(c == 0),
                                stop=(c == NKC - 1),
                            )
                            nc.tensor.matmul(
                                avO[:],
                                vh[:, c, 65:130],
                                pts[c][:, 1, :],
                                start=(c == 0),
                                stop=(c == NKC - 1),
                            )
                        for parity, av in ((0, avE), (1, avO)):
                            # denominator -> reciprocal (rebased to partition 0)
                            rdc = rd_p.tile([1, TLOC], F32, tag="rdc")
                            nc.vector.tensor_copy(rdc[:], av[64:65, :])
                            rd = rd_p.tile([1, TLOC], F32, tag="rd")
                            nc.vector.reciprocal(rd[:], rdc[:])
                            if b == 1:
                                nc.vector.tensor_scalar_mul(
                                    rd[:], rd[:], lam_sb[0:1, 0:1]
                                )
                            rdd = rdd_pool.tile([1, TLOC], F32, tag="rdd")
                            nc.sync.dma_start(rdd[:], rd[:])
                            rows = slice(parity * 64, parity * 64 + 64)
                            rdb = rdb_p.tile([128, TLOC], F32, tag="rdb")
                            nc.sync.dma_start(rdb[rows, :], _bcast_part(rdd[:], 64))
                            if b == 0:
                                nc.vector.tensor_mul(
                                    o1[rows, :], av[0:64, :], rdb[rows, :]
                                )
                            else:
                                o2 = rdb_p.tile([128, TLOC], F32, tag="o2")
                                nc.vector.tensor_mul(
                                    o2[rows, :], av[0:64, :], rdb[rows, :]
                                )
                                nc.vector.tensor_sub(
                                    attn_sb[rows, hp, :], o1[rows, :], o2[rows, :]
                                )

            # ---------------- Phase 3: output projection ----------------
            with (
                tc.tile_pool(name="wo_p", bufs=KT) as wo_p,
                tc.tile_pool(name="ye_p", bufs=3) as ye_p,
                tc.tile_pool(name="yp", bufs=2, space="PSUM") as yp,
            ):
                wostrips = []
                for j in range(KT):
                    ws = wo_p.tile([128, D], DT, tag="wo", name=f"w_o{j}")
                    nc.sync.dma_start(ws[:], wo_t[j * 128 : (j + 1) * 128, :])
                    wostrips.append(ws)
                for dt_ in range(D // 128):
                    yps = yp.tile([128, TLOC], F32, tag="y")
                    for j in range(KT):
                        nc.tensor.matmul(
                            yps[:],
                            wostrips[j][:, dt_ * 128 : (dt_ + 1) * 128],
                            attn_sb[:, j, :],
                            start=(j == 0),
                            stop=(j == KT - 1),
                        )
                    ye = ye_p.tile([128, TLOC], F32, tag="ye")
                    nc.vector.tensor_copy(ye[:], yps[:])
                    nc.sync.dma_start(yT[dt_ * 128 : (dt_ + 1) * 128, :], ye[:])

    nc.compile()
    return nc


# ---------------- host-side preparation ----------------


def _quantize(W):
    W = np.asarray(W, dtype=np.float32)
    scale = np.clip(np.abs(W).mean(axis=1, keepdims=True), 1e-5, None)
    wq = np.clip(np.round(W / scale), -1.0, 1.0)
    return (wq * scale).astype(np.float32)


def prepare_inputs(
    x, Wq, Wk, Wv, Wo, lambda_q, lambda_k, qn_gamma, qn_beta, kn_gamma, kn_beta,
    mm_dt=MM_DT,
):
    """Host prep: quantize + center weights, fold gamma, per-core slices."""
    np_dt = mybir.dt.np(_DT_MAP[mm_dt])
    x = np.asarray(x, dtype=np.float32)
    t_total = x.shape[1]
    ch = 2 * H * DH
    cht = ch // 128
    tloc = t_total // GS

    Wq_e = _quantize(Wq)
    Wk_e = _quantize(Wk)
    Wv_e = _quantize(Wv)
    Wo_e = _quantize(Wo)
    # fold LN mean-subtraction into column-centered weights, gamma into rows
    gq = np.asarray(qn_gamma, np.float32)
    gk = np.asarray(kn_gamma, np.float32)
    Wq_c = (Wq_e - Wq_e.mean(axis=0, keepdims=True)) * gq[:, None]
    Wk_c = (Wk_e - Wk_e.mean(axis=0, keepdims=True)) * gk[:, None]

    wq_t = np.ascontiguousarray(Wq_c.T).astype(np_dt)
    wk_t = np.ascontiguousarray(Wk_c.T).astype(np_dt)
    wv_t = np.ascontiguousarray(Wv_e.T).astype(np_dt)
    wo_t = np.ascontiguousarray(Wo_e.T).astype(np_dt)

    def wsq_full(g):
        # stationary for the variance matmul: [128, cht*128] where column
        # block t is constant per-partition 1/(ch * gamma[t*128+p]^2)
        w = 1.0 / (ch * np.maximum(g, 1e-12) ** 2)
        return np.ascontiguousarray(
            np.repeat(w.reshape(cht, 128).T[:, :, None], 128, axis=2).reshape(
                128, cht * 128
            )
        ).astype(np_dt)

    lam = np.clip(
        np.exp(np.asarray(lambda_q).mean() - np.asarray(lambda_k).mean()), 0.1, 2.0
    ).astype(np.float32)

    has_beta = bool(np.any(np.asarray(qn_beta)) or np.any(np.asarray(kn_beta)))
    common = {
        "wq_t": wq_t,
        "wk_t": wk_t,
        "wv_t": wv_t,
        "wo_t": wo_t,
        "wsq_q": wsq_full(gq),
        "wsq_k": wsq_full(gk),
        "lam": lam.reshape(1, 1),
        "ones_one": np.ones((128, H), np_dt),
    }
    if has_beta:
        bq = np.asarray(qn_beta, np.float32) * (DH**-0.5)
        bk = np.asarray(kn_beta, np.float32)
        common["gs_k"] = np.ascontiguousarray(gk.reshape(cht, 128).T)
        common["bq"] = np.ascontiguousarray(bq.reshape(cht, 128).T)
        common["bk"] = np.ascontiguousarray(bk.reshape(cht, 128).T)

    in_maps = []
    for c in range(NCORES):
        b = c // GS
        ts = c % GS
        xt = np.ascontiguousarray(x[b, ts * tloc : (ts + 1) * tloc, :].T).astype(np_dt)
        in_maps.append({**common, "xT": xt})
    return in_maps, has_beta, t_total


def get_program(t_total=T, has_beta=False, mm_dt=MM_DT):
    key = (t_total, has_beta, mm_dt)
    if key not in _PROG_CACHE:
        _PROG_CACHE[key] = build_program(t_total, has_beta, mm_dt)
    return _PROG_CACHE[key]


def run(inputs, trace=False, mm_dt=MM_DT):
    """Run on hardware; returns (full_output, BassKernelResults)."""
    in_maps, has_beta, t_total = prepare_inputs(**inputs, mm_dt=mm_dt)
    nc = get_program(t_total, has_beta, mm_dt)
    res = run_bass_kernel_spmd(nc, in_maps, list(range(NCORES)), trace=trace)
    tloc = t_total // GS
    out = np.empty((B, t_total, D), dtype=np.float32)
    for c in range(NCORES):
        b = c // GS
        ts = c % GS
        out[b, ts * tloc : (ts + 1) * tloc, :] = res.results[c]["yT"].T
    return out, res


def kernel(**inputs) -> np.ndarray:
    out, _ = run(inputs, trace=False)
    return out


# revision 13
# speedup vs baseline: 1.9469x; 1.2097x over previous
"""DifferentialAttention Trainium2 kernel (8 NeuronCores, SPMD).

Sharding: 2 batches x 4 token-blocks = 8 cores. Each core computes
projections + layernorm for its 512 tokens, AllGathers K/V within its
4-core (same-batch) group, then computes attention + output projection
for its own query tokens. Host stitches per-core [D, 512] outputs.

Layout strategy: everything "transposed" (features on partitions, tokens
on free dim) so the whole chain q-proj -> scores -> AV -> out-proj needs
zero on-device transposes:
  - LN mean subtraction is folded into host-side column-centering of the
    (ternary-quantized) weight matrices, and gamma is folded into the
    weight rows; only the variance needs device work (a matmul of the
    squared activations against a per-channel 1/(CH*gamma^2) stationary).
  - K's 1/std never touches K on device: it rides as the per-partition
    `scale` operand of the softmax exp (scoresT has keys on partitions),
    so raw centered K can be gathered as soon as its projection finishes.
  - softmax runs without max-subtraction (scores are O(+-8); exp is safe)
    and the denominator comes for free as a 65th "ones" column appended
    to V.
  - lambda (a host-computable scalar) rides in as a [1,1] input tensor so
    the compiled program is input-independent.

Attention is branch-outer, and each (branch, head-pair) runs all 16
score-matmuls + exps first, then all 32 AV matmuls — the PE's in-order
queue never waits on the ScalarE exp stream.

MM_DT picks the matmul operand dtype: float16 (default), bfloat16, or
float32r.
"""

import os
import sys
import types

for _p in ("/opt/trn_rl_repo",):
    if os.path.isdir(_p) and _p not in sys.path:
        sys.path.append(_p)

import numpy as np

import concourse.bass as bass
import concourse.tile as tile
from concourse.bass import _add_dep_helper
from concourse import bacc, mybir
from concourse.bass_utils import run_bass_kernel_spmd


def _install_ntff_shim():
    """bass_utils imports antenv.axon_hooks when tracing under axon; the
    container antenv stub lacks it. Back it with the ctypes hook."""
    if "antenv.axon_hooks" in sys.modules:
        return
    try:
        from trn_agent_boot.trn_boot import _ntff_profile_via_ctypes

        hook = _ntff_profile_via_ctypes("/opt/axon/libaxon_pjrt.so")
    except Exception:
        hook = None
    mod = types.ModuleType("antenv.axon_hooks")
    mod.get_axon_ntff_profile_hook = lambda: hook
    sys.modules["antenv.axon_hooks"] = mod


_install_ntff_shim()

# ----- problem dims (hardcoded per spec) -----
B, T, D = 2, 2048, 1024
H, DH = 16, 64
CH = 2 * H * DH  # 2048
EPS = 1e-5
NCORES = 8
GS = 4  # cores per batch group
GROUPS = [[0, 1, 2, 3], [4, 5, 6, 7]]

F32 = mybir.dt.float32
MM_DT = "f16"  # "f16" | "bf16" | "f32r"
_DT_MAP = {
    "f16": mybir.dt.float16,
    "bf16": mybir.dt.bfloat16,
    "f32r": mybir.dt.float32r,
}

_PROG_CACHE: dict = {}


def _bcast_part(ap, n):
    """AP view replicating a 1-partition AP across n partitions (step 0)."""
    return bass.AP(tensor=ap.tensor, offset=ap.offset, ap=[[0, n]] + list(ap.ap)[1:])


def build_program(t_total=T, has_beta=False, mm_dt=MM_DT):
    """Build the per-core SPMD program. t_total is the per-batch sequence
    length (2048 for the real problem; smaller for simulation)."""
    TLOC = t_total // GS  # query tokens owned by this core
    KT = D // 128  # contraction tiles for projections
    CHT = CH // 128  # channel tiles of q/k
    NKC = t_total // 128  # key chunks
    TC = TLOC // 128  # token chunks (v projection)
    L = TLOC // 128  # 128-blocks per gather block
    HP = H // 2  # head pairs
    VW = H * 65  # augmented v width (64 + ones column per head)
    DT = _DT_MAP[mm_dt]
    fast_k = not has_beta  # beta on K forces the on-device LN-apply path

    nc = bacc.Bacc("TRN2", target_bir_lowering=False, debug=False, num_devices=NCORES)

    xT = nc.dram_tensor("xT", [D, TLOC], DT, kind="ExternalInput").ap()
    wq_t = nc.dram_tensor("wq_t", [D, CH], DT, kind="ExternalInput").ap()
    wk_t = nc.dram_tensor("wk_t", [D, CH], DT, kind="ExternalInput").ap()
    wv_t = nc.dram_tensor("wv_t", [D, D], DT, kind="ExternalInput").ap()
    wo_t = nc.dram_tensor("wo_t", [D, D], DT, kind="ExternalInput").ap()
    wsq_q = nc.dram_tensor("wsq_q", [128, CH], DT, kind="ExternalInput").ap()
    wsq_k = nc.dram_tensor("wsq_k", [128, CH], DT, kind="ExternalInput").ap()
    lam_in = nc.dram_tensor("lam", [1, 1], F32, kind="ExternalInput").ap()
    ones_one_in = nc.dram_tensor("ones_one", [128, H], DT, kind="ExternalInput").ap()
    if has_beta:
        gs_k_in = nc.dram_tensor("gs_k", [128, CHT], F32, kind="ExternalInput").ap()
        bq_in = nc.dram_tensor("bq", [128, CHT], F32, kind="ExternalInput").ap()
        bk_in = nc.dram_tensor("bk", [128, CHT], F32, kind="ExternalInput").ap()
    yT = nc.dram_tensor("yT", [D, TLOC], F32, kind="ExternalOutput").ap()

    with tile.TileContext(nc) as tc:
        with (
            tc.tile_pool(name="const", bufs=1) as const,
            tc.tile_pool(name="dram", bufs=1, space="DRAM") as dram,
            tc.tile_pool(name="rdd_pool", bufs=4, space="DRAM") as rdd_pool,
            tc.tile_pool(name="qf_p", bufs=1) as qf_p,
            tc.tile_pool(name="attn_p", bufs=1) as attn_p,
        ):
            # constants + tiny inputs
            ones_one = const.tile([128, H], DT)
            nc.sync.dma_start(ones_one[:], ones_one_in[:])
            lam_sb = const.tile([1, 1], F32)
            nc.sync.dma_start(lam_sb[:], lam_in[:])
            wsq_q_sb = const.tile([128, CH], DT)
            nc.sync.dma_start(wsq_q_sb[:], wsq_q[:])
            wsq_k_sb = const.tile([128, CH], DT)
            nc.sync.dma_start(wsq_k_sb[:], wsq_k[:])
            gsk_sb = bq_sb = bk_sb = None
            if has_beta:
                gsk_sb = const.tile([128, CHT], F32)
                nc.sync.dma_start(gsk_sb[:], gs_k_in[:])
                bq_sb = const.tile([128, CHT], F32)
                nc.sync.dma_start(bq_sb[:], bq_in[:])
                bk_sb = const.tile([128, CHT], F32)
                nc.sync.dma_start(bk_sb[:], bk_in[:])

            # DRAM bounce/gather buffers. kT_loc0 carries branch-0 K in
            # rows 0..CH/2-1 plus rstd_k as row CH/2 (rides the same gather).
            KR0 = CH // 2 + 2 if fast_k else CH // 2
            kT_loc0 = dram.tile([KR0, TLOC], DT)
            kT_loc1 = dram.tile([CH // 2, TLOC], DT)
            kT_g0 = dram.tile([GS, KR0, TLOC], DT)
            kT_g1 = dram.tile([GS, CH // 2, TLOC], DT)
            vA_loc0 = dram.tile([TLOC, VW // 2], DT)
            vA_loc1 = dram.tile([TLOC, VW // 2], DT)
            vA_g0 = dram.tile([GS, TLOC, VW // 2], DT)
            vA_g1 = dram.tile([GS, TLOC, VW // 2], DT)

            # ---------------- Phase 1: projections + LN + gathers -------
            with (
                tc.tile_pool(name="xp", bufs=1) as xp,
                tc.tile_pool(name="w_p", bufs=KT) as w_p,
                tc.tile_pool(name="prime_p", bufs=1) as prime_p,
                tc.tile_pool(name="sq_p", bufs=3) as sq_p,
                tc.tile_pool(name="stat_p", bufs=2) as stat_p,
                tc.tile_pool(name="ev_p", bufs=3) as ev_p,
                tc.tile_pool(name="vaug_p", bufs=2) as vaug_p,
                tc.tile_pool(name="pp", bufs=1, space="PSUM") as pp,
            ):
                xT_sb = xp.tile([128, KT, TLOC], DT)
                nc.sync.dma_start(xT_sb[:], xT.rearrange("(k p) t -> p k t", p=128))

                def load_strips(pool, dram_t, width, tag):
                    # all strips share one slot-cycled pool sized [128, width]
                    strips = []
                    dw = dram_t.shape[1]
                    for j in range(KT):
                        ws = pool.tile([128, width], DT, tag=tag, name=f"w_{tag}{j}")
                        nc.sync.dma_start(
                            ws[:, 0:dw], dram_t[j * 128 : (j + 1) * 128, :]
                        )
                        strips.append(ws)
                    return strips

                # ---- K projection (first: its gather gates attention) ----
                kstrips = load_strips(w_p, wk_t, CH, "w")
                kprime = None
                if not fast_k:
                    kprime = prime_p.tile([128, CHT, TLOC], F32, tag="kprime")
                k_var = pp.tile([128, TLOC], F32, tag="var", bufs=2)
                for t in range(CHT):
                    ps = pp.tile([128, TLOC], F32, tag="proj", bufs=2)
                    for j in range(KT):
                        nc.tensor.matmul(
                            ps[:],
                            kstrips[j][:, t * 128 : (t + 1) * 128],
                            xT_sb[:, j, :],
                            start=(j == 0),
                            stop=(j == KT - 1),
                        )
                    sq = sq_p.tile([128, TLOC], DT, tag="sq")
                    if fast_k:
                        kf = ev_p.tile([128, TLOC], DT, tag="kf", name=f"kf{t}")
                        nc.vector.tensor_copy(kf[:], ps[:])
                        nc.scalar.square(sq[:], ps[:])
                        if t < CHT // 2:
                            nc.sync.dma_start(
                                kT_loc0[t * 128 : (t + 1) * 128, :], kf[:]
                            )
                        else:
                            tt = t - CHT // 2
                            nc.sync.dma_start(
                                kT_loc1[tt * 128 : (tt + 1) * 128, :], kf[:]
                            )
                    else:
                        nc.vector.tensor_copy(kprime[:, t, :], ps[:])
                        nc.scalar.square(sq[:], ps[:])
                    nc.tensor.matmul(
                        k_var[:],
                        wsq_k_sb[:, t * 128 : (t + 1) * 128],
                        sq[:],
                        start=(t == 0),
                        stop=(t == CHT - 1),
                    )

                if fast_k:
                    # rstd_k: slim [1, TLOC] chain; rides row CH/2 of kT_loc0
                    kv1 = stat_p.tile([1, TLOC], F32, tag="kv1")
                    nc.vector.tensor_scalar_add(kv1[:], k_var[0:1, :], EPS)
                    kv2 = stat_p.tile([1, TLOC], F32, tag="kv2")
                    nc.vector.reciprocal(kv2[:], kv1[:])
                    kv3 = stat_p.tile([1, TLOC], F32, tag="kv3")
                    nc.scalar.sqrt(kv3[:], kv2[:])
                    # split into hi+lo fp16 parts (sum reconstructs ~21
                    # mantissa bits; a bare fp16 rstd perturbs logits by up to
                    # |score| * 2^-11 which is too coarse)
                    kv3d = stat_p.tile([1, TLOC], DT, tag="kv3d")
                    nc.vector.tensor_copy(kv3d[:], kv3[:])
                    kv3dF = stat_p.tile([1, TLOC], F32, tag="kv3dF")
                    nc.vector.tensor_copy(kv3dF[:], kv3d[:])
                    kv3lo = stat_p.tile([1, TLOC], DT, tag="kv3lo")
                    nc.vector.tensor_sub(kv3lo[:], kv3[:], kv3dF[:])
                    nc.sync.dma_start(kT_loc0[CH // 2 : CH // 2 + 1, :], kv3d[:])
                    nc.sync.dma_start(kT_loc0[CH // 2 + 1 : CH // 2 + 2, :], kv3lo[:])
                else:
                    veps = stat_p.tile([128, TLOC], F32, tag="veps")
                    nc.vector.tensor_scalar_add(veps[:], k_var[:], EPS)
                    rec = stat_p.tile([128, TLOC], F32, tag="rec")
                    nc.vector.reciprocal(rec[:], veps[:])
                    rstd = stat_p.tile([128, TLOC], F32, tag="rstd")
                    nc.scalar.sqrt(rstd[:], rec[:])
                    for t in range(CHT):
                        kf = ev_p.tile([128, TLOC], DT, tag="kf", name=f"kfs{t}")
                        # gamma is already folded into the weights host-side
                        nc.vector.scalar_tensor_tensor(
                            out=kf[:],
                            in0=kprime[:, t, :],
                            scalar=1.0,
                            in1=rstd[:],
                            op0=mybir.AluOpType.mult,
                            op1=mybir.AluOpType.mult,
                        )
                        nc.vector.tensor_scalar_add(kf[:], kf[:], bk_sb[:, t : t + 1])
                        if t < CHT // 2:
                            nc.sync.dma_start(
                                kT_loc0[t * 128 : (t + 1) * 128, :], kf[:]
                            )
                        else:
                            tt = t - CHT // 2
                            nc.sync.dma_start(
                                kT_loc1[tt * 128 : (tt + 1) * 128, :], kf[:]
                            )

                # gathers: k0(+rstd), v, k1 — explicitly chained so the
                # scheduler cannot reorder them on the cc queue
                cc_k0 = nc.gpsimd.collective_compute(
                    "AllGather",
                    mybir.AluOpType.bypass,
                    replica_groups=GROUPS,
                    ins=[kT_loc0[:]],
                    outs=[kT_g0[:]],
                )

                # ---- V projection ----
                vstrips = load_strips(w_p, wv_t, CH, "w")
                for c in range(TC):
                    vps = pp.tile([128, D], F32, tag="vproj", bufs=2)
                    for j in range(KT):
                        for n in range(D // 512):
                            nc.tensor.matmul(
                                vps[:, n * 512 : (n + 1) * 512],
                                xT_sb[:, j, c * 128 : (c + 1) * 128],
                                vstrips[j][:, n * 512 : (n + 1) * 512],
                                start=(j == 0),
                                stop=(j == KT - 1),
                            )
                    vaug = vaug_p.tile([128, H, 65], DT, tag="vaug")
                    nc.vector.tensor_copy(
                        vaug[:, :, 0:64], vps[:].rearrange("p (h d) -> p h d", h=H)
                    )
                    nc.sync.dma_start(vaug[:, :, 64:65], ones_one[:])
                    vflat = vaug.rearrange("p h d -> p (h d)")
                    nc.sync.dma_start(
                        vA_loc0[c * 128 : (c + 1) * 128, :], vflat[:, 0 : VW // 2]
                    )
                    nc.sync.dma_start(
                        vA_loc1[c * 128 : (c + 1) * 128, :],
                        vflat[:, VW // 2 : VW],
                    )
                cc_v0 = nc.gpsimd.collective_compute(
                    "AllGather",
                    mybir.AluOpType.bypass,
                    replica_groups=GROUPS,
                    ins=[vA_loc0[:]],
                    outs=[vA_g0[:]],
                )
                _add_dep_helper(cc_v0.ins, cc_k0.ins, sync=True, reason="cc order")
                cc_k1 = nc.gpsimd.collective_compute(
                    "AllGather",
                    mybir.AluOpType.bypass,
                    replica_groups=GROUPS,
                    ins=[kT_loc1[:]],
                    outs=[kT_g1[:]],
                )
                _add_dep_helper(cc_k1.ins, cc_v0.ins, sync=True, reason="cc order")
                cc_v1 = nc.gpsimd.collective_compute(
                    "AllGather",
                    mybir.AluOpType.bypass,
                    replica_groups=GROUPS,
                    ins=[vA_loc1[:]],
                    outs=[vA_g1[:]],
                )
                _add_dep_helper(cc_v1.ins, cc_k1.ins, sync=True, reason="cc order")

                # ---- Q projection (stays resident; LN applied on device) ----
                qstrips = load_strips(w_p, wq_t, CH, "w")
                qf_sb = qf_p.tile([128, CHT, TLOC], DT)
                qprime = prime_p.tile([128, CHT, TLOC], F32, tag="qprime")
                q_var = pp.tile([128, TLOC], F32, tag="var", bufs=2)
                for t in range(CHT):
                    ps = pp.tile([128, TLOC], F32, tag="proj", bufs=2)
                    for j in range(KT):
                        nc.tensor.matmul(
                            ps[:],
                            qstrips[j][:, t * 128 : (t + 1) * 128],
                            xT_sb[:, j, :],
                            start=(j == 0),
                            stop=(j == KT - 1),
                        )
                    nc.vector.tensor_copy(qprime[:, t, :], ps[:])
                    sq = sq_p.tile([128, TLOC], DT, tag="sq")
                    nc.scalar.square(sq[:], ps[:])
                    nc.tensor.matmul(
                        q_var[:],
                        wsq_q_sb[:, t * 128 : (t + 1) * 128],
                        sq[:],
                        start=(t == 0),
                        stop=(t == CHT - 1),
                    )
                veps = stat_p.tile([128, TLOC], F32, tag="veps")
                nc.vector.tensor_scalar_add(veps[:], q_var[:], EPS)
                rec = stat_p.tile([128, TLOC], F32, tag="rec")
                nc.vector.reciprocal(rec[:], veps[:])
                rstd = stat_p.tile([128, TLOC], F32, tag="rstd")
                nc.scalar.sqrt(rstd[:], rec[:])
                for t in range(CHT):
                    nc.vector.scalar_tensor_tensor(
                        out=qf_sb[:, t, :],
                        in0=qprime[:, t, :],
                        scalar=DH**-0.5,
                        in1=rstd[:],
                        op0=mybir.AluOpType.mult,
                        op1=mybir.AluOpType.mult,
                    )
                    if has_beta:
                        nc.vector.tensor_scalar_add(
                            qf_sb[:, t, :], qf_sb[:, t, :], bq_sb[:, t : t + 1]
                        )

            # ---------------- Phase 2: attention ------------------------
            with (
                tc.tile_pool(name="rk_p", bufs=1) as rk_p,
                tc.tile_pool(name="kh_p", bufs=4) as kh_p,
                tc.tile_pool(name="vh_p", bufs=3) as vh_p,
                tc.tile_pool(name="pt_p", bufs=2 * NKC + 2) as pt_p,
                tc.tile_pool(name="o1_p", bufs=HP) as o1_p,
                tc.tile_pool(name="rd_p", bufs=4) as rd_p,
                tc.tile_pool(name="rdb_p", bufs=4) as rdb_p,
                tc.tile_pool(name="scp", bufs=2, space="PSUM") as scp,
                tc.tile_pool(name="avp", bufs=4, space="PSUM") as avp,
            ):
                attn_sb = attn_p.tile([128, HP, TLOC], DT)
                if fast_k:
                    rk_hi = rk_p.tile([128, NKC], DT)
                    rk_lo = rk_p.tile([128, NKC], DT)
                    for g in range(GS):
                        nc.sync.dma_start(
                            rk_hi[:, g * L : (g + 1) * L],
                            kT_g0[g, CH // 2, :].rearrange("(l p) -> p l", p=128),
                        )
                        nc.sync.dma_start(
                            rk_lo[:, g * L : (g + 1) * L],
                            kT_g0[g, CH // 2 + 1, :].rearrange("(l p) -> p l", p=128),
                        )
                    rk_hiF = rk_p.tile([128, NKC], F32)
                    nc.vector.tensor_copy(rk_hiF[:], rk_hi[:])
                    rk_loF = rk_p.tile([128, NKC], F32)
                    nc.vector.tensor_copy(rk_loF[:], rk_lo[:])
                    rk_sb = rk_p.tile([128, NKC], F32)
                    nc.vector.tensor_add(rk_sb[:], rk_hiF[:], rk_loF[:])
                o1_tiles = {}

                def combine(st):
                    bb, hpp, avs = st
                    o1 = o1_tiles[hpp]
                    for parity, av in ((0, avs[0]), (1, avs[1])):
                        rdc = rd_p.tile([1, TLOC], F32, tag="rdc")
                        nc.vector.tensor_copy(rdc[:], av[64:65, :])
                        rd = rd_p.tile([1, TLOC], F32, tag="rd")
                        nc.vector.reciprocal(rd[:], rdc[:])
                        if bb == 1:
                            nc.vector.tensor_scalar_mul(
                                rd[:], rd[:], lam_sb[0:1, 0:1]
                            )
                        rdd = rdd_pool.tile([1, TLOC], F32, tag="rdd")
                        nc.sync.dma_start(rdd[:], rd[:])
                        rows = slice(parity * 64, parity * 64 + 64)
                        rdb = rdb_p.tile([128, TLOC], F32, tag="rdb")
                        nc.sync.dma_start(rdb[rows, :], _bcast_part(rdd[:], 64))
                        if bb == 0:
                            nc.vector.tensor_mul(
                                o1[rows, :], av[0:64, :], rdb[rows, :]
                            )
                        else:
                            o2 = rdb_p.tile([128, TLOC], F32, tag="o2")
                            nc.vector.tensor_mul(
                                o2[rows, :], av[0:64, :], rdb[rows, :]
                            )
                            nc.vector.tensor_sub(
                                attn_sb[rows, hpp, :], o1[rows, :], o2[rows, :]
                            )

                pairs = [(b, hp) for b in range(2) for hp in range(HP)]
                prev = None  # (b, hp, (avE, avO), pts, vh)
                for b, hp in pairs:
                    vA_gb = vA_g0 if hp < HP // 2 else vA_g1
                    hpo = hp if hp < HP // 2 else hp - HP // 2
                    vh = vh_p.tile([128, NKC, 130], DT, tag="vh")
                    for g in range(GS):
                        nc.sync.dma_start(
                            vh[:, g * L : (g + 1) * L, :],
                            vA_gb[
                                g, :, (2 * hpo) * 65 : (2 * hpo + 2) * 65
                            ].rearrange("(l p) c -> p l c", p=128),
                        )
                    kT_gb = kT_g0 if b == 0 else kT_g1
                    kh = kh_p.tile([128, t_total], DT, tag="kh")
                    for g in range(GS):
                        nc.sync.dma_start(
                            kh[:, g * TLOC : (g + 1) * TLOC],
                            kT_gb[g, hp * 128 : (hp + 1) * 128, :],
                        )
                    if b == 0:
                        o1 = o1_p.tile([128, TLOC], F32, tag="o1", name=f"o1_{hp}")
                        o1_tiles[hp] = o1
                    qE = qf_sb[0:64, b * (CHT // 2) + hp, :]
                    qO = qf_sb[64:128, b * (CHT // 2) + hp, :]
                    # scores + exp for (b, hp), interleaved with the AV
                    # matmuls of the previous iteration: the in-order PE
                    # queue always has ready work and never waits on ACT.
                    pav = None
                    if prev is not None:
                        pav = (
                            avp.tile([65, TLOC], F32, tag="av", name="pavE"),
                            avp.tile([65, TLOC], F32, tag="av", name="pavO"),
                        )
                    pts = []
                    for c in range(NKC):
                        sc = scp.tile([128, 2, TLOC], F32, tag="sc")
                        nc.tensor.matmul(
                            sc[:, 0, :],
                            kh[0:64, c * 128 : (c + 1) * 128],
                            qE,
                            start=True,
                            stop=True,
                        )
                        nc.tensor.matmul(
                            sc[:, 1, :],
                            kh[64:128, c * 128 : (c + 1) * 128],
                            qO,
                            start=True,
                            stop=True,
                        )
                        pt = pt_p.tile([128, 2, TLOC], DT, tag="pt")
                        if fast_k:
                            nc.scalar.activation(
                                pt[:],
                                sc[:],
                                mybir.ActivationFunctionType.Exp,
                                scale=rk_sb[:, c : c + 1],
                            )
                        else:
                            nc.scalar.activation(
                                pt[:], sc[:], mybir.ActivationFunctionType.Exp
                            )
                        pts.append(pt)
                        if prev is not None:
                            pb, php, ppts, pvh = prev
                            nc.tensor.matmul(
                                pav[0][:],
                                pvh[:, c, 0:65],
                                ppts[c][:, 0, :],
                                start=(c == 0),
                                stop=(c == NKC - 1),
                            )
                            nc.tensor.matmul(
                                pav[1][:],
                                pvh[:, c, 65:130],
                                ppts[c][:, 1, :],
                                start=(c == 0),
                                stop=(c == NKC - 1),
                            )
                    if prev is not None:
                        combine((prev[0], prev[1], pav))
                    prev = (b, hp, pts, vh)
                # flush the last iteration's AV + combine
                lb, lhp, lpts, lvh = prev
                lav = (
                    avp.tile([65, TLOC], F32, tag="av", name="lavE"),
                    avp.tile([65, TLOC], F32, tag="av", name="lavO"),
                )
                for c in range(NKC):
                    nc.tensor.matmul(
                        lav[0][:],
                        lvh[:, c, 0:65],
                        lpts[c][:, 0, :],
                        start=(c == 0),
                        stop=(c == NKC - 1),
                    )
                    nc.tensor.matmul(
                        lav[1][:],
                        lvh[:, c, 65:130],
                        lpts[c][:, 1, :],
                        start=(c == 0),
                        stop=(c == NKC - 1),
                    )
                combine((lb, lhp, lav))

            # ---------------- Phase 3: output projection ----------------
            with (
                tc.tile_pool(name="wo_p", bufs=KT) as wo_p,
                tc.tile_pool(name="ye_p", bufs=3) as ye_p,
                tc.tile_pool(name="yp", bufs=2, space="PSUM") as yp,
            ):
                wostrips = []
                for j in range(KT):
                    ws = wo_p.tile([128, D], DT, tag="wo", name=f"w_o{j}")
                    nc.sync.dma_start(ws[:], wo_t[j * 128 : (j + 1) * 128, :])
                    wostrips.append(ws)
                for dt_ in range(D // 128):
                    yps = yp.tile([128, TLOC], F32, tag="y")
                    for j in range(KT):
                        nc.tensor.matmul(
                            yps[:],
                            wostrips[j][:, dt_ * 128 : (dt_ + 1) * 128],
                            attn_sb[:, j, :],
                            start=(j == 0),
                            stop=(j == KT - 1),
                        )
                    ye = ye_p.tile([128, TLOC], F32, tag="ye")
                    nc.vector.tensor_copy(ye[:], yps[:])
                    nc.sync.dma_start(yT[dt_ * 128 : (dt_ + 1) * 128, :], ye[:])

    nc.compile()
    return nc


# ---------------- host-side preparation ----------------


def _quantize(W):
    W = np.asarray(W, dtype=np.float32)
    scale = np.clip(np.abs(W).mean(axis=1, keepdims=True), 1e-5, None)
    wq = np.clip(np.round(W / scale), -1.0, 1.0)
    return (wq * scale).astype(np.float32)


def prepare_inputs(
    x, Wq, Wk, Wv, Wo, lambda_q, lambda_k, qn_gamma, qn_beta, kn_gamma, kn_beta,
    mm_dt=MM_DT,
):
    """Host prep: quantize + center weights, fold gamma, per-core slices."""
    np_dt = mybir.dt.np(_DT_MAP[mm_dt])
    x = np.asarray(x, dtype=np.float32)
    t_total = x.shape[1]
    ch = 2 * H * DH
    cht = ch // 128
    tloc = t_total // GS

    Wq_e = _quantize(Wq)
    Wk_e = _quantize(Wk)
    Wv_e = _quantize(Wv)
    Wo_e = _quantize(Wo)
    # fold LN mean-subtraction into column-centered weights, gamma into rows
    gq = np.asarray(qn_gamma, np.float32)
    gk = np.asarray(kn_gamma, np.float32)
    Wq_c = (Wq_e - Wq_e.mean(axis=0, keepdims=True)) * gq[:, None]
    Wk_c = (Wk_e - Wk_e.mean(axis=0, keepdims=True)) * gk[:, None]

    wq_t = np.ascontiguousarray(Wq_c.T).astype(np_dt)
    wk_t = np.ascontiguousarray(Wk_c.T).astype(np_dt)
    wv_t = np.ascontiguousarray(Wv_e.T).astype(np_dt)
    wo_t = np.ascontiguousarray(Wo_e.T).astype(np_dt)

    def wsq_full(g):
        # stationary for the variance matmul: [128, cht*128] where column
        # block t is constant per-partition 1/(ch * gamma[t*128+p]^2)
        w = 1.0 / (ch * np.maximum(g, 1e-12) ** 2)
        return np.ascontiguousarray(
            np.repeat(w.reshape(cht, 128).T[:, :, None], 128, axis=2).reshape(
                128, cht * 128
            )
        ).astype(np_dt)

    lam = np.clip(
        np.exp(np.asarray(lambda_q).mean() - np.asarray(lambda_k).mean()), 0.1, 2.0
    ).astype(np.float32)

    has_beta = bool(np.any(np.asarray(qn_beta)) or np.any(np.asarray(kn_beta)))
    common = {
        "wq_t": wq_t,
        "wk_t": wk_t,
        "wv_t": wv_t,
        "wo_t": wo_t,
        "wsq_q": wsq_full(gq),
        "wsq_k": wsq_full(gk),
        "lam": lam.reshape(1, 1),
        "ones_one": np.ones((128, H), np_dt),
    }
    if has_beta:
        bq = np.asarray(qn_beta, np.float32) * (DH**-0.5)
        bk = np.asarray(kn_beta, np.float32)
        common["gs_k"] = np.ascontiguousarray(gk.reshape(cht, 128).T)
        common["bq"] = np.ascontiguousarray(bq.reshape(cht, 128).T)
        common["bk"] = np.ascontiguousarray(bk.reshape(cht, 128).T)

    in_maps = []
    for c in range(NCORES):
        b = c // GS
        ts = c % GS
        xt = np.ascontiguousarray(x[b, ts * tloc : (ts + 1) * tloc, :].T).astype(np_dt)
        in_maps.append({**common, "xT": xt})
    return in_maps, has_beta, t_total


def get_program(t_total=T, has_beta=False, mm_dt=MM_DT):
    key = (t_total, has_beta, mm_dt)
    if key not in _PROG_CACHE:
        _PROG_CACHE[key] = build_program(t_total, has_beta, mm_dt)
    return _PROG_CACHE[key]


def run(inputs, trace=False, mm_dt=MM_DT):
    """Run on hardware; returns (full_output, BassKernelResults)."""
    in_maps, has_beta, t_total = prepare_inputs(**inputs, mm_dt=mm_dt)
    nc = get_program(t_total, has_beta, mm_dt)
    res = run_bass_kernel_spmd(nc, in_maps, list(range(NCORES)), trace=trace)
    tloc = t_total // GS
    out = np.empty((B, t_total, D), dtype=np.float32)
    for c in range(NCORES):
        b = c // GS
        ts = c % GS
        out[b, ts * tloc : (ts + 1) * tloc, :] = res.results[c]["yT"].T
    return out, res


def kernel(**inputs) -> np.ndarray:
    out, _ = run(inputs, trace=False)
    return out


# revision 14
# speedup vs baseline: 1.9521x; 1.0027x over previous
"""DifferentialAttention Trainium2 kernel (8 NeuronCores, SPMD).

Sharding: 2 batches x 4 token-blocks = 8 cores. Each core computes
projections + layernorm for its 512 tokens, AllGathers K/V within its
4-core (same-batch) group, then computes attention + output projection
for its own query tokens. Host stitches per-core [D, 512] outputs.

Layout strategy: everything "transposed" (features on partitions, tokens
on free dim) so the whole chain q-proj -> scores -> AV -> out-proj needs
zero on-device transposes:
  - LN mean subtraction is folded into host-side column-centering of the
    (ternary-quantized) weight matrices, and gamma is folded into the
    weight rows; only the variance needs device work (a matmul of the
    squared activations against a per-channel 1/(CH*gamma^2) stationary).
  - K's 1/std never touches K on device: it rides as the per-partition
    `scale` operand of the softmax exp (scoresT has keys on partitions),
    so raw centered K can be gathered as soon as its projection finishes.
  - softmax runs without max-subtraction (scores are O(+-8); exp is safe)
    and the denominator comes for free as a 65th "ones" column appended
    to V.
  - lambda (a host-computable scalar) rides in as a [1,1] input tensor so
    the compiled program is input-independent.

Attention is branch-outer, and each (branch, head-pair) runs all 16
score-matmuls + exps first, then all 32 AV matmuls — the PE's in-order
queue never waits on the ScalarE exp stream.

MM_DT picks the matmul operand dtype: float16 (default), bfloat16, or
float32r.
"""

import os
import sys
import types

for _p in ("/opt/trn_rl_repo",):
    if os.path.isdir(_p) and _p not in sys.path:
        sys.path.append(_p)

import numpy as np

import concourse.bass as bass
import concourse.tile as tile
from concourse.bass import _add_dep_helper
from concourse import bacc, mybir
from concourse.bass_utils import run_bass_kernel_spmd


def _install_ntff_shim():
    """bass_utils imports antenv.axon_hooks when tracing under axon; the
    container antenv stub lacks it. Back it with the ctypes hook."""
    if "antenv.axon_hooks" in sys.modules:
        return
    try:
        from trn_agent_boot.trn_boot import _ntff_profile_via_ctypes

        hook = _ntff_profile_via_ctypes("/opt/axon/libaxon_pjrt.so")
    except Exception:
        hook = None
    mod = types.ModuleType("antenv.axon_hooks")
    mod.get_axon_ntff_profile_hook = lambda: hook
    sys.modules["antenv.axon_hooks"] = mod


_install_ntff_shim()

# ----- problem dims (hardcoded per spec) -----
B, T, D = 2, 2048, 1024
H, DH = 16, 64
CH = 2 * H * DH  # 2048
EPS = 1e-5
NCORES = 8
GS = 4  # cores per batch group
GROUPS = [[0, 1, 2, 3], [4, 5, 6, 7]]

F32 = mybir.dt.float32
MM_DT = "f16"  # "f16" | "bf16" | "f32r"
_DT_MAP = {
    "f16": mybir.dt.float16,
    "bf16": mybir.dt.bfloat16,
    "f32r": mybir.dt.float32r,
}

_PROG_CACHE: dict = {}


def _bcast_part(ap, n):
    """AP view replicating a 1-partition AP across n partitions (step 0)."""
    return bass.AP(tensor=ap.tensor, offset=ap.offset, ap=[[0, n]] + list(ap.ap)[1:])


def build_program(t_total=T, has_beta=False, mm_dt=MM_DT):
    """Build the per-core SPMD program. t_total is the per-batch sequence
    length (2048 for the real problem; smaller for simulation)."""
    TLOC = t_total // GS  # query tokens owned by this core
    KT = D // 128  # contraction tiles for projections
    CHT = CH // 128  # channel tiles of q/k
    NKC = t_total // 128  # key chunks
    TC = TLOC // 128  # token chunks (v projection)
    L = TLOC // 128  # 128-blocks per gather block
    HP = H // 2  # head pairs
    VW = H * 65  # augmented v width (64 + ones column per head)
    DT = _DT_MAP[mm_dt]
    fast_k = not has_beta  # beta on K forces the on-device LN-apply path

    nc = bacc.Bacc("TRN2", target_bir_lowering=False, debug=False, num_devices=NCORES)

    xT = nc.dram_tensor("xT", [D, TLOC], DT, kind="ExternalInput").ap()
    wq_t = nc.dram_tensor("wq_t", [D, CH], DT, kind="ExternalInput").ap()
    wk_t = nc.dram_tensor("wk_t", [D, CH], DT, kind="ExternalInput").ap()
    wv_t = nc.dram_tensor("wv_t", [D, D], DT, kind="ExternalInput").ap()
    wo_t = nc.dram_tensor("wo_t", [D, D], DT, kind="ExternalInput").ap()
    wsq_q = nc.dram_tensor("wsq_q", [128, CH], DT, kind="ExternalInput").ap()
    wsq_k = nc.dram_tensor("wsq_k", [128, CH], DT, kind="ExternalInput").ap()
    lam_in = nc.dram_tensor("lam", [1, 1], F32, kind="ExternalInput").ap()
    ones_one_in = nc.dram_tensor("ones_one", [128, H], DT, kind="ExternalInput").ap()
    if has_beta:
        gs_k_in = nc.dram_tensor("gs_k", [128, CHT], F32, kind="ExternalInput").ap()
        bq_in = nc.dram_tensor("bq", [128, CHT], F32, kind="ExternalInput").ap()
        bk_in = nc.dram_tensor("bk", [128, CHT], F32, kind="ExternalInput").ap()
    yT = nc.dram_tensor("yT", [D, TLOC], F32, kind="ExternalOutput").ap()

    with tile.TileContext(nc) as tc:
        with (
            tc.tile_pool(name="const", bufs=1) as const,
            tc.tile_pool(name="dram", bufs=1, space="DRAM") as dram,
            tc.tile_pool(name="rdd_pool", bufs=4, space="DRAM") as rdd_pool,
            tc.tile_pool(name="qf_p", bufs=1) as qf_p,
            tc.tile_pool(name="attn_p", bufs=1) as attn_p,
        ):
            # constants + tiny inputs
            ones_one = const.tile([128, H], DT)
            nc.sync.dma_start(ones_one[:], ones_one_in[:])
            lam_sb = const.tile([1, 1], F32)
            nc.sync.dma_start(lam_sb[:], lam_in[:])
            wsq_q_sb = const.tile([128, CH], DT)
            nc.sync.dma_start(wsq_q_sb[:], wsq_q[:])
            wsq_k_sb = const.tile([128, CH], DT)
            nc.sync.dma_start(wsq_k_sb[:], wsq_k[:])
            gsk_sb = bq_sb = bk_sb = None
            if has_beta:
                gsk_sb = const.tile([128, CHT], F32)
                nc.sync.dma_start(gsk_sb[:], gs_k_in[:])
                bq_sb = const.tile([128, CHT], F32)
                nc.sync.dma_start(bq_sb[:], bq_in[:])
                bk_sb = const.tile([128, CHT], F32)
                nc.sync.dma_start(bk_sb[:], bk_in[:])

            # DRAM bounce/gather buffers. kT_loc0 carries branch-0 K in
            # rows 0..CH/2-1 plus rstd_k as row CH/2 (rides the same gather).
            KR0 = CH // 4 + 2 if fast_k else CH // 4
            kT_loc0a = dram.tile([KR0, TLOC], DT)
            kT_loc0b = dram.tile([CH // 4, TLOC], DT)
            kT_loc1 = dram.tile([CH // 2, TLOC], DT)
            kT_g0a = dram.tile([GS, KR0, TLOC], DT)
            kT_g0b = dram.tile([GS, CH // 4, TLOC], DT)
            kT_g1 = dram.tile([GS, CH // 2, TLOC], DT)
            vA_loc0 = dram.tile([TLOC, VW // 2], DT)
            vA_loc1 = dram.tile([TLOC, VW // 2], DT)
            vA_g0 = dram.tile([GS, TLOC, VW // 2], DT)
            vA_g1 = dram.tile([GS, TLOC, VW // 2], DT)

            # ---------------- Phase 1: projections + LN + gathers -------
            with (
                tc.tile_pool(name="xp", bufs=1) as xp,
                tc.tile_pool(name="w_p", bufs=KT) as w_p,
                tc.tile_pool(name="prime_p", bufs=1) as prime_p,
                tc.tile_pool(name="sq_p", bufs=3) as sq_p,
                tc.tile_pool(name="stat_p", bufs=2) as stat_p,
                tc.tile_pool(name="ev_p", bufs=3) as ev_p,
                tc.tile_pool(name="vaug_p", bufs=2) as vaug_p,
                tc.tile_pool(name="pp", bufs=1, space="PSUM") as pp,
            ):
                xT_sb = xp.tile([128, KT, TLOC], DT)
                nc.sync.dma_start(xT_sb[:], xT.rearrange("(k p) t -> p k t", p=128))

                def load_strips(pool, dram_t, width, tag):
                    # all strips share one slot-cycled pool sized [128, width]
                    strips = []
                    dw = dram_t.shape[1]
                    for j in range(KT):
                        ws = pool.tile([128, width], DT, tag=tag, name=f"w_{tag}{j}")
                        nc.sync.dma_start(
                            ws[:, 0:dw], dram_t[j * 128 : (j + 1) * 128, :]
                        )
                        strips.append(ws)
                    return strips

                # ---- K projection (first: its gather gates attention) ----
                kstrips = load_strips(w_p, wk_t, CH, "w")
                kprime = None
                if not fast_k:
                    kprime = prime_p.tile([128, CHT, TLOC], F32, tag="kprime")
                k_var = pp.tile([128, TLOC], F32, tag="var", bufs=2)
                for t in range(CHT):
                    ps = pp.tile([128, TLOC], F32, tag="proj", bufs=2)
                    for j in range(KT):
                        nc.tensor.matmul(
                            ps[:],
                            kstrips[j][:, t * 128 : (t + 1) * 128],
                            xT_sb[:, j, :],
                            start=(j == 0),
                            stop=(j == KT - 1),
                        )
                    sq = sq_p.tile([128, TLOC], DT, tag="sq")
                    if fast_k:
                        kf = ev_p.tile([128, TLOC], DT, tag="kf", name=f"kf{t}")
                        nc.vector.tensor_copy(kf[:], ps[:])
                        nc.scalar.square(sq[:], ps[:])
                        if t < CHT // 4:
                            nc.sync.dma_start(
                                kT_loc0a[t * 128 : (t + 1) * 128, :], kf[:]
                            )
                        elif t < CHT // 2:
                            tt = t - CHT // 4
                            nc.sync.dma_start(
                                kT_loc0b[tt * 128 : (tt + 1) * 128, :], kf[:]
                            )
                        else:
                            tt = t - CHT // 2
                            nc.sync.dma_start(
                                kT_loc1[tt * 128 : (tt + 1) * 128, :], kf[:]
                            )
                    else:
                        nc.vector.tensor_copy(kprime[:, t, :], ps[:])
                        nc.scalar.square(sq[:], ps[:])
                    nc.tensor.matmul(
                        k_var[:],
                        wsq_k_sb[:, t * 128 : (t + 1) * 128],
                        sq[:],
                        start=(t == 0),
                        stop=(t == CHT - 1),
                    )

                if fast_k:
                    # rstd_k: slim [1, TLOC] chain; rides row CH/2 of kT_loc0
                    kv1 = stat_p.tile([1, TLOC], F32, tag="kv1")
                    nc.vector.tensor_scalar_add(kv1[:], k_var[0:1, :], EPS)
                    kv2 = stat_p.tile([1, TLOC], F32, tag="kv2")
                    nc.vector.reciprocal(kv2[:], kv1[:])
                    kv3 = stat_p.tile([1, TLOC], F32, tag="kv3")
                    nc.scalar.sqrt(kv3[:], kv2[:])
                    # split into hi+lo fp16 parts (sum reconstructs ~21
                    # mantissa bits; a bare fp16 rstd perturbs logits by up to
                    # |score| * 2^-11 which is too coarse)
                    kv3d = stat_p.tile([1, TLOC], DT, tag="kv3d")
                    nc.vector.tensor_copy(kv3d[:], kv3[:])
                    kv3dF = stat_p.tile([1, TLOC], F32, tag="kv3dF")
                    nc.vector.tensor_copy(kv3dF[:], kv3d[:])
                    kv3lo = stat_p.tile([1, TLOC], DT, tag="kv3lo")
                    nc.vector.tensor_sub(kv3lo[:], kv3[:], kv3dF[:])
                    nc.sync.dma_start(
                        kT_loc0a[CH // 4 : CH // 4 + 1, :], kv3d[:]
                    )
                    nc.sync.dma_start(
                        kT_loc0a[CH // 4 + 1 : CH // 4 + 2, :], kv3lo[:]
                    )
                else:
                    veps = stat_p.tile([128, TLOC], F32, tag="veps")
                    nc.vector.tensor_scalar_add(veps[:], k_var[:], EPS)
                    rec = stat_p.tile([128, TLOC], F32, tag="rec")
                    nc.vector.reciprocal(rec[:], veps[:])
                    rstd = stat_p.tile([128, TLOC], F32, tag="rstd")
                    nc.scalar.sqrt(rstd[:], rec[:])
                    for t in range(CHT):
                        kf = ev_p.tile([128, TLOC], DT, tag="kf", name=f"kfs{t}")
                        # gamma is already folded into the weights host-side
                        nc.vector.scalar_tensor_tensor(
                            out=kf[:],
                            in0=kprime[:, t, :],
                            scalar=1.0,
                            in1=rstd[:],
                            op0=mybir.AluOpType.mult,
                            op1=mybir.AluOpType.mult,
                        )
                        nc.vector.tensor_scalar_add(kf[:], kf[:], bk_sb[:, t : t + 1])
                        if t < CHT // 4:
                            nc.sync.dma_start(
                                kT_loc0a[t * 128 : (t + 1) * 128, :], kf[:]
                            )
                        elif t < CHT // 2:
                            tt = t - CHT // 4
                            nc.sync.dma_start(
                                kT_loc0b[tt * 128 : (tt + 1) * 128, :], kf[:]
                            )
                        else:
                            tt = t - CHT // 2
                            nc.sync.dma_start(
                                kT_loc1[tt * 128 : (tt + 1) * 128, :], kf[:]
                            )

                # gathers: k0(+rstd), v, k1 — explicitly chained so the
                # scheduler cannot reorder them on the cc queue
                cc_k0 = nc.gpsimd.collective_compute(
                    "AllGather",
                    mybir.AluOpType.bypass,
                    replica_groups=GROUPS,
                    ins=[kT_loc0a[:]],
                    outs=[kT_g0a[:]],
                )

                # ---- Q projection (stays resident; LN applied on device) ----
                qstrips = load_strips(w_p, wq_t, CH, "w")
                qf_sb = qf_p.tile([128, CHT, TLOC], DT)
                qprime = prime_p.tile([128, CHT, TLOC], F32, tag="qprime")
                q_var = pp.tile([128, TLOC], F32, tag="var", bufs=2)
                for t in range(CHT):
                    ps = pp.tile([128, TLOC], F32, tag="proj", bufs=2)
                    for j in range(KT):
                        nc.tensor.matmul(
                            ps[:],
                            qstrips[j][:, t * 128 : (t + 1) * 128],
                            xT_sb[:, j, :],
                            start=(j == 0),
                            stop=(j == KT - 1),
                        )
                    nc.vector.tensor_copy(qprime[:, t, :], ps[:])
                    sq = sq_p.tile([128, TLOC], DT, tag="sq")
                    nc.scalar.square(sq[:], ps[:])
                    nc.tensor.matmul(
                        q_var[:],
                        wsq_q_sb[:, t * 128 : (t + 1) * 128],
                        sq[:],
                        start=(t == 0),
                        stop=(t == CHT - 1),
                    )
                veps = stat_p.tile([128, TLOC], F32, tag="veps")
                nc.vector.tensor_scalar_add(veps[:], q_var[:], EPS)
                rec = stat_p.tile([128, TLOC], F32, tag="rec")
                nc.vector.reciprocal(rec[:], veps[:])
                rstd = stat_p.tile([128, TLOC], F32, tag="rstd")
                nc.scalar.sqrt(rstd[:], rec[:])
                for t in range(CHT):
                    nc.vector.scalar_tensor_tensor(
                        out=qf_sb[:, t, :],
                        in0=qprime[:, t, :],
                        scalar=DH**-0.5,
                        in1=rstd[:],
                        op0=mybir.AluOpType.mult,
                        op1=mybir.AluOpType.mult,
                    )
                    if has_beta:
                        nc.vector.tensor_scalar_add(
                            qf_sb[:, t, :], qf_sb[:, t, :], bq_sb[:, t : t + 1]
                        )

                # ---- V projection ----
                vstrips = load_strips(w_p, wv_t, CH, "w")
                for c in range(TC):
                    vps = pp.tile([128, D], F32, tag="vproj", bufs=2)
                    for j in range(KT):
                        for n in range(D // 512):
                            nc.tensor.matmul(
                                vps[:, n * 512 : (n + 1) * 512],
                                xT_sb[:, j, c * 128 : (c + 1) * 128],
                                vstrips[j][:, n * 512 : (n + 1) * 512],
                                start=(j == 0),
                                stop=(j == KT - 1),
                            )
                    vaug = vaug_p.tile([128, H, 65], DT, tag="vaug")
                    nc.vector.tensor_copy(
                        vaug[:, :, 0:64], vps[:].rearrange("p (h d) -> p h d", h=H)
                    )
                    nc.sync.dma_start(vaug[:, :, 64:65], ones_one[:])
                    vflat = vaug.rearrange("p h d -> p (h d)")
                    nc.sync.dma_start(
                        vA_loc0[c * 128 : (c + 1) * 128, :], vflat[:, 0 : VW // 2]
                    )
                    nc.sync.dma_start(
                        vA_loc1[c * 128 : (c + 1) * 128, :],
                        vflat[:, VW // 2 : VW],
                    )
                cc_v0 = nc.gpsimd.collective_compute(
                    "AllGather",
                    mybir.AluOpType.bypass,
                    replica_groups=GROUPS,
                    ins=[vA_loc0[:]],
                    outs=[vA_g0[:]],
                )
                _add_dep_helper(cc_v0.ins, cc_k0.ins, sync=True, reason="cc order")
                cc_k0b = nc.gpsimd.collective_compute(
                    "AllGather",
                    mybir.AluOpType.bypass,
                    replica_groups=GROUPS,
                    ins=[kT_loc0b[:]],
                    outs=[kT_g0b[:]],
                )
                _add_dep_helper(cc_k0b.ins, cc_v0.ins, sync=True, reason="cc order")
                cc_k1 = nc.gpsimd.collective_compute(
                    "AllGather",
                    mybir.AluOpType.bypass,
                    replica_groups=GROUPS,
                    ins=[kT_loc1[:]],
                    outs=[kT_g1[:]],
                )
                _add_dep_helper(cc_k1.ins, cc_k0b.ins, sync=True, reason="cc order")
                cc_v1 = nc.gpsimd.collective_compute(
                    "AllGather",
                    mybir.AluOpType.bypass,
                    replica_groups=GROUPS,
                    ins=[vA_loc1[:]],
                    outs=[vA_g1[:]],
                )
                _add_dep_helper(cc_v1.ins, cc_k1.ins, sync=True, reason="cc order")

            # ---------------- Phase 2: attention ------------------------
            with (
                tc.tile_pool(name="rk_p", bufs=1) as rk_p,
                tc.tile_pool(name="kh_p", bufs=4) as kh_p,
                tc.tile_pool(name="vh_p", bufs=3) as vh_p,
                tc.tile_pool(name="pt_p", bufs=2 * NKC + 2) as pt_p,
                tc.tile_pool(name="o1_p", bufs=HP) as o1_p,
                tc.tile_pool(name="rd_p", bufs=4) as rd_p,
                tc.tile_pool(name="rdb_p", bufs=4) as rdb_p,
                tc.tile_pool(name="scp", bufs=2, space="PSUM") as scp,
                tc.tile_pool(name="avp", bufs=4, space="PSUM") as avp,
            ):
                attn_sb = attn_p.tile([128, HP, TLOC], DT)
                if fast_k:
                    rk_hi = rk_p.tile([128, NKC], DT)
                    rk_lo = rk_p.tile([128, NKC], DT)
                    for g in range(GS):
                        nc.sync.dma_start(
                            rk_hi[:, g * L : (g + 1) * L],
                            kT_g0a[g, CH // 4, :].rearrange("(l p) -> p l", p=128),
                        )
                        nc.sync.dma_start(
                            rk_lo[:, g * L : (g + 1) * L],
                            kT_g0a[g, CH // 4 + 1, :].rearrange(
                                "(l p) -> p l", p=128
                            ),
                        )
                    rk_hiF = rk_p.tile([128, NKC], F32)
                    nc.vector.tensor_copy(rk_hiF[:], rk_hi[:])
                    rk_loF = rk_p.tile([128, NKC], F32)
                    nc.vector.tensor_copy(rk_loF[:], rk_lo[:])
                    rk_sb = rk_p.tile([128, NKC], F32)
                    nc.vector.tensor_add(rk_sb[:], rk_hiF[:], rk_loF[:])
                o1_tiles = {}

                def combine(st):
                    bb, hpp, avs = st
                    o1 = o1_tiles[hpp]
                    for parity, av in ((0, avs[0]), (1, avs[1])):
                        rdc = rd_p.tile([1, TLOC], F32, tag="rdc")
                        nc.vector.tensor_copy(rdc[:], av[64:65, :])
                        rd = rd_p.tile([1, TLOC], F32, tag="rd")
                        nc.vector.reciprocal(rd[:], rdc[:])
                        if bb == 1:
                            nc.vector.tensor_scalar_mul(
                                rd[:], rd[:], lam_sb[0:1, 0:1]
                            )
                        rdd = rdd_pool.tile([1, TLOC], F32, tag="rdd")
                        nc.sync.dma_start(rdd[:], rd[:])
                        rows = slice(parity * 64, parity * 64 + 64)
                        rdb = rdb_p.tile([128, TLOC], F32, tag="rdb")
                        nc.sync.dma_start(rdb[rows, :], _bcast_part(rdd[:], 64))
                        if bb == 0:
                            nc.vector.tensor_mul(
                                o1[rows, :], av[0:64, :], rdb[rows, :]
                            )
                        else:
                            o2 = rdb_p.tile([128, TLOC], F32, tag="o2")
                            nc.vector.tensor_mul(
                                o2[rows, :], av[0:64, :], rdb[rows, :]
                            )
                            nc.vector.tensor_sub(
                                attn_sb[rows, hpp, :], o1[rows, :], o2[rows, :]
                            )

                pairs = [(b, hp) for b in range(2) for hp in range(HP)]
                prev = None  # (b, hp, (avE, avO), pts, vh)
                for b, hp in pairs:
                    vA_gb = vA_g0 if hp < HP // 2 else vA_g1
                    hpo = hp if hp < HP // 2 else hp - HP // 2
                    vh = vh_p.tile([128, NKC, 130], DT, tag="vh")
                    for g in range(GS):
                        nc.sync.dma_start(
                            vh[:, g * L : (g + 1) * L, :],
                            vA_gb[
                                g, :, (2 * hpo) * 65 : (2 * hpo + 2) * 65
                            ].rearrange("(l p) c -> p l c", p=128),
                        )
                    if b == 0:
                        kT_gb = kT_g0a if hp < HP // 2 else kT_g0b
                        khp = hp if hp < HP // 2 else hp - HP // 2
                    else:
                        kT_gb, khp = kT_g1, hp
                    kh = kh_p.tile([128, t_total], DT, tag="kh")
                    for g in range(GS):
                        nc.sync.dma_start(
                            kh[:, g * TLOC : (g + 1) * TLOC],
                            kT_gb[g, khp * 128 : (khp + 1) * 128, :],
                        )
                    if b == 0:
                        o1 = o1_p.tile([128, TLOC], F32, tag="o1", name=f"o1_{hp}")
                        o1_tiles[hp] = o1
                    qE = qf_sb[0:64, b * (CHT // 2) + hp, :]
                    qO = qf_sb[64:128, b * (CHT // 2) + hp, :]
                    # scores + exp for (b, hp), interleaved with the AV
                    # matmuls of the previous iteration: the in-order PE
                    # queue always has ready work and never waits on ACT.
                    pav = None
                    if prev is not None:
                        pav = (
                            avp.tile([65, TLOC], F32, tag="av", name="pavE"),
                            avp.tile([65, TLOC], F32, tag="av", name="pavO"),
                        )
                    pts = []
                    for c in range(NKC):
                        sc = scp.tile([128, 2, TLOC], F32, tag="sc")
                        nc.tensor.matmul(
                            sc[:, 0, :],
                            kh[0:64, c * 128 : (c + 1) * 128],
                            qE,
                            start=True,
                            stop=True,
                        )
                        nc.tensor.matmul(
                            sc[:, 1, :],
                            kh[64:128, c * 128 : (c + 1) * 128],
                            qO,
                            start=True,
                            stop=True,
                        )
                        pt = pt_p.tile([128, 2, TLOC], DT, tag="pt")
                        if fast_k:
                            nc.scalar.activation(
                                pt[:],
                                sc[:],
                                mybir.ActivationFunctionType.Exp,
                                scale=rk_sb[:, c : c + 1],
                            )
                        else:
                            nc.scalar.activation(
                                pt[:], sc[:], mybir.ActivationFunctionType.Exp
                            )
                        pts.append(pt)
                        if prev is not None:
                            pb, php, ppts, pvh = prev
                            nc.tensor.matmul(
                                pav[0][:],
                                pvh[:, c, 0:65],
                                ppts[c][:, 0, :],
                                start=(c == 0),
                                stop=(c == NKC - 1),
                            )
                            nc.tensor.matmul(
                                pav[1][:],
                                pvh[:, c, 65:130],
                                ppts[c][:, 1, :],
                                start=(c == 0),
                                stop=(c == NKC - 1),
                            )
                    if prev is not None:
                        combine((prev[0], prev[1], pav))
                    prev = (b, hp, pts, vh)
                # flush the last iteration's AV + combine
                lb, lhp, lpts, lvh = prev
                lav = (
                    avp.tile([65, TLOC], F32, tag="av", name="lavE"),
                    avp.tile([65, TLOC], F32, tag="av", name="lavO"),
                )
                for c in range(NKC):
                    nc.tensor.matmul(
                        lav[0][:],
                        lvh[:, c, 0:65],
                        lpts[c][:, 0, :],
                        start=(c == 0),
                        stop=(c == NKC - 1),
                    )
                    nc.tensor.matmul(
                        lav[1][:],
                        lvh[:, c, 65:130],
                        lpts[c][:, 1, :],
                        start=(c == 0),
                        stop=(c == NKC - 1),
                    )
                combine((lb, lhp, lav))

            # ---------------- Phase 3: output projection ----------------
            with (
                tc.tile_pool(name="wo_p", bufs=KT) as wo_p,
                tc.tile_pool(name="ye_p", bufs=3) as ye_p,
                tc.tile_pool(name="yp", bufs=2, space="PSUM") as yp,
            ):
                wostrips = []
                for j in range(KT):
                    ws = wo_p.tile([128, D], DT, tag="wo", name=f"w_o{j}")
                    nc.sync.dma_start(ws[:], wo_t[j * 128 : (j + 1) * 128, :])
                    wostrips.append(ws)
                for dt_ in range(D // 128):
                    yps = yp.tile([128, TLOC], F32, tag="y")
                    for j in range(KT):
                        nc.tensor.matmul(
                            yps[:],
                            wostrips[j][:, dt_ * 128 : (dt_ + 1) * 128],
                            attn_sb[:, j, :],
                            start=(j == 0),
                            stop=(j == KT - 1),
                        )
                    ye = ye_p.tile([128, TLOC], F32, tag="ye")
                    nc.vector.tensor_copy(ye[:], yps[:])
                    nc.sync.dma_start(yT[dt_ * 128 : (dt_ + 1) * 128, :], ye[:])

    nc.compile()
    return nc


# ---------------- host-side preparation ----------------


def _quantize(W):
    W = np.asarray(W, dtype=np.float32)
    scale = np.clip(np.abs(W).mean(axis=1, keepdims=True), 1e-5, None)
    wq = np.clip(np.round(W / scale), -1.0, 1.0)
    return (wq * scale).astype(np.float32)


def prepare_inputs(
    x, Wq, Wk, Wv, Wo, lambda_q, lambda_k, qn_gamma, qn_beta, kn_gamma, kn_beta,
    mm_dt=MM_DT,
):
    """Host prep: quantize + center weights, fold gamma, per-core slices."""
    np_dt = mybir.dt.np(_DT_MAP[mm_dt])
    x = np.asarray(x, dtype=np.float32)
    t_total = x.shape[1]
    ch = 2 * H * DH
    cht = ch // 128
    tloc = t_total // GS

    Wq_e = _quantize(Wq)
    Wk_e = _quantize(Wk)
    Wv_e = _quantize(Wv)
    Wo_e = _quantize(Wo)
    # fold LN mean-subtraction into column-centered weights, gamma into rows
    gq = np.asarray(qn_gamma, np.float32)
    gk = np.asarray(kn_gamma, np.float32)
    Wq_c = (Wq_e - Wq_e.mean(axis=0, keepdims=True)) * gq[:, None]
    Wk_c = (Wk_e - Wk_e.mean(axis=0, keepdims=True)) * gk[:, None]

    wq_t = np.ascontiguousarray(Wq_c.T).astype(np_dt)
    wk_t = np.ascontiguousarray(Wk_c.T).astype(np_dt)
    wv_t = np.ascontiguousarray(Wv_e.T).astype(np_dt)
    wo_t = np.ascontiguousarray(Wo_e.T).astype(np_dt)

    def wsq_full(g):
        # stationary for the variance matmul: [128, cht*128] where column
        # block t is constant per-partition 1/(ch * gamma[t*128+p]^2)
        w = 1.0 / (ch * np.maximum(g, 1e-12) ** 2)
        return np.ascontiguousarray(
            np.repeat(w.reshape(cht, 128).T[:, :, None], 128, axis=2).reshape(
                128, cht * 128
            )
        ).astype(np_dt)

    lam = np.clip(
        np.exp(np.asarray(lambda_q).mean() - np.asarray(lambda_k).mean()), 0.1, 2.0
    ).astype(np.float32)

    has_beta = bool(np.any(np.asarray(qn_beta)) or np.any(np.asarray(kn_beta)))
    common = {
        "wq_t": wq_t,
        "wk_t": wk_t,
        "wv_t": wv_t,
        "wo_t": wo_t,
        "wsq_q": wsq_full(gq),
        "wsq_k": wsq_full(gk),
        "lam": lam.reshape(1, 1),
        "ones_one": np.ones((128, H), np_dt),
    }
    if has_beta:
        bq = np.asarray(qn_beta, np.float32) * (DH**-0.5)
        bk = np.asarray(kn_beta, np.float32)
        common["gs_k"] = np.ascontiguousarray(gk.reshape(cht, 128).T)
        common["bq"] = np.ascontiguousarray(bq.reshape(cht, 128).T)
        common["bk"] = np.ascontiguousarray(bk.reshape(cht, 128).T)

    in_maps = []
    for c in range(NCORES):
        b = c // GS
        ts = c % GS
        xt = np.ascontiguousarray(x[b, ts * tloc : (ts + 1) * tloc, :].T).astype(np_dt)
        in_maps.append({**common, "xT": xt})
    return in_maps, has_beta, t_total


def get_program(t_total=T, has_beta=False, mm_dt=MM_DT):
    key = (t_total, has_beta, mm_dt)
    if key not in _PROG_CACHE:
        _PROG_CACHE[key] = build_program(t_total, has_beta, mm_dt)
    return _PROG_CACHE[key]


def run(inputs, trace=False, mm_dt=MM_DT):
    """Run on hardware; returns (full_output, BassKernelResults)."""
    in_maps, has_beta, t_total = prepare_inputs(**inputs, mm_dt=mm_dt)
    nc = get_program(t_total, has_beta, mm_dt)
    res = run_bass_kernel_spmd(nc, in_maps, list(range(NCORES)), trace=trace)
    tloc = t_total // GS
    out = np.empty((B, t_total, D), dtype=np.float32)
    for c in range(NCORES):
        b = c // GS
        ts = c % GS
        out[b, ts * tloc : (ts + 1) * tloc, :] = res.results[c]["yT"].T
    return out, res


def kernel(**inputs) -> np.ndarray:
    out, _ = run(inputs, trace=False)
    return out
